# revision 14
# baseline (speedup 1.0000x reference)
"""Trainium2 Bass kernel for nn_APDIntelligibilityEstimator.

Model: audio encoder conv(k=40,s=20) -> GroupNorm(1)+PReLU -> two 1x1
BitConv (fused into one 512x512 int matmul on host) -> 24 depthwise-separable
TCN blocks (dconv k=3 dil 2^(i%8) -> GN+PReLU -> 1x1 BitConv -> residual)
-> global avg pool -> BitLinear -> PReLU -> BitLinear -> sigmoid.

Sharding: data-parallel over batch, 2 samples per core on 8 NeuronCores.

Host-side weight preprocessing exploits exact algebraic identities of the
reference (x_scale cancellation in bit ops; composition of the two 1x1 sign
convs into one integer matrix; folding of w_scale*scale into the GN
affine+PReLU pass).

Device mapping per TCN layer:
  - depthwise side taps (t-d, t+d) -> TensorE diagonal matmuls into PSUM
  - center tap + combine           -> DVE scalar_tensor_tensor from PSUM
  - sum(g) for GN mean             -> edge-corrected from residual accum_out
                                      (no full pass)
  - sum(g^2) for GN var            -> ScalarE Square pass with accum_out
  - GN affine + PReLU + w_scale    -> one ScalarE Prelu pass (per-channel
                                      scale/bias APs)
  - 1x1 conv (+-1 weights, bf16)   -> TensorE matmuls
  - residual add + pooled sums     -> DVE scalar_tensor_tensor with accum_out
"""

import hashlib
import os
import sys

import numpy as np

sys.path.insert(0, os.path.dirname(os.path.abspath(__file__)))

N_TCN = 24
DILATIONS = [2 ** (i % 8) for i in range(N_TCN)]
B, T_IN = 16, 64000
T = 3201           # conv output length
TE = 3202          # allocated elementwise width (col T stays zero)
PAD = 128          # max dilation; zero padding on both sides of f
FW = PAD + TE + PAD
C = 512
KC = 4             # channel chunks of 128
CT = C * T
EPS = 1e-5
NCORES = 8
SPC = 2            # samples per core
PW = 1536          # psum macro-tile width (3 banks)
NT_SPANS = [(0, 1536), (1536, 1536), (3072, 129)]

N_LAYERS = int(os.environ.get("K_NLAYERS", str(N_TCN)))  # debug knob
SKIP_ENC = os.environ.get("K_SKIPENC", "0") == "1"        # debug knob
K_DEBUG = os.environ.get("K_DEBUG", "0") == "1"           # debug knob
K_SUMSQ = os.environ.get("K_SUMSQ", "act")                # act | dve
K_RESID = os.environ.get("K_RESID", "dve")                # dve | act


def _f32(a):
    return np.ascontiguousarray(a, dtype=np.float32)


def _bf16(a):
    import ml_dtypes

    return np.ascontiguousarray(np.asarray(a, dtype=np.float32).astype(ml_dtypes.bfloat16))


def _prep(inp):
    """Host-side weight preprocessing. Returns (x, arrays, baked_scalars)."""
    x = np.asarray(inp["x"], dtype=np.float32)          # (16,1,64000)
    enc_w = np.asarray(inp["enc_w"], dtype=np.float32)  # (512,1,40)
    ew = enc_w[:, 0, :]
    encw = np.stack([ew[:, :20].T, ew[:, 20:].T])        # (2,20,512)

    sign = np.sign
    bn_w = np.asarray(inp["bn_w"], dtype=np.float32)[:, :, 0]    # (256,512)
    ti_w = np.asarray(inp["ti_w"], dtype=np.float32)[:, :, 0]    # (512,256)
    wbt = (sign(ti_w) @ sign(bn_w)).T                            # (512,512) ints
    cbt = float(
        np.abs(bn_w).mean() * inp["bn_scale"][0] * np.abs(ti_w).mean() * inp["ti_scale"][0]
    )

    pw_w = np.asarray(inp["pw_w"], dtype=np.float32)[:, :, :, 0]  # (24,512,512)
    c_pw = np.abs(pw_w).mean(axis=(1, 2)) * np.asarray(inp["pw_scale"], dtype=np.float32)[:, 0]
    assert np.all(c_pw > 0)
    # fold w_scale*scale into the +-1 weights (+-c, exact-ish in bf16)
    pwt = (sign(pw_w) * c_pw[:, None, None]).transpose(0, 2, 1)   # (24,512,512) [in,out]

    dw_w = np.asarray(inp["dw_w"], dtype=np.float32)[:, :, 0, :]  # (24,512,3)
    dwc = dw_w.reshape(24, KC, 128, 3).transpose(2, 0, 1, 3)      # (128,24,4,3)
    # diagonal weight matrices for side taps on TensorE: (24, 2, KC, 128, 128)
    dwdiag = np.zeros((N_TCN, 2, KC, 128, 128), dtype=np.float32)
    idx = np.arange(128)
    for i in range(N_TCN):
        for jt, tap in enumerate((0, 2)):
            for k in range(KC):
                dwdiag[i, jt, k, idx, idx] = dw_w[i, k * 128 : (k + 1) * 128, tap]

    tg = np.asarray(inp["tcn_gamma"], dtype=np.float32)  # (24,512)
    tb = np.asarray(inp["tcn_beta"], dtype=np.float32)
    gbf = np.stack(
        [
            tg.reshape(24, KC, 128).transpose(2, 0, 1),
            tb.reshape(24, KC, 128).transpose(2, 0, 1),
        ],
        axis=-1,
    )  # (128,24,4,2)

    encgb = np.stack(
        [
            np.asarray(inp["enc_gamma"], dtype=np.float32).reshape(KC, 128).T,
            np.asarray(inp["enc_beta"], dtype=np.float32).reshape(KC, 128).T,
        ],
        axis=-1,
    )  # (128,4,2)

    alph = np.concatenate(
        [
            np.atleast_1d(np.float32(inp["enc_prelu"])),
            np.asarray(inp["tcn_prelu"], dtype=np.float32),
            np.atleast_1d(np.float32(inp["head_prelu"])),
        ]
    )[None, :]  # (1,26)

    fc1_w = np.asarray(inp["fc1_w"], dtype=np.float32)   # (256,512)
    c1 = float(np.abs(fc1_w).mean() * inp["fc1_scale"][0])
    w1t = sign(fc1_w).T                                   # (512,256)
    b1c = (np.asarray(inp["fc1_b"], dtype=np.float32) * c1).reshape(2, 128).T  # (128,2)

    fco_w = np.asarray(inp["fco_w"], dtype=np.float32)   # (1,256)
    c2 = float(np.abs(fco_w).mean() * inp["fco_scale"][0])
    w2t = sign(fco_w).T                                   # (256,1)
    b2c = float(np.asarray(inp["fco_b"], dtype=np.float32)[0] * c2)

    arrays = dict(
        encw=_bf16(encw),
        wbt=_bf16(wbt),
        pwt=_bf16(pwt),
        dwdiag=_bf16(dwdiag),
        dwc=_f32(dwc),
        gbf=_f32(gbf),
        encgb=_f32(encgb),
        alph=_f32(alph),
        w1t=_bf16(w1t),
        w2t=_bf16(w2t),
        b1c=_f32(b1c),
    )
    baked = (cbt, c1, b2c, float(c2))
    return x, arrays, baked


# ---------------------------------------------------------------------------

_CACHE = {}


def _build(baked):
    import concourse.bacc as bacc
    import concourse.bass as bass
    import concourse.mybir as mybir
    import concourse.tile as tile

    cbt, c1, b2c, c2 = baked
    f32 = mybir.dt.float32
    bf16 = mybir.dt.bfloat16
    AF = mybir.ActivationFunctionType
    OP = mybir.AluOpType
    AX = mybir.AxisListType

    nc = bacc.Bacc("TRN2", target_bir_lowering=False, debug=False)

    x2 = nc.dram_tensor("x2", [SPC, T_IN], f32, kind="ExternalInput").ap()
    encw = nc.dram_tensor("encw", [2, 20, C], bf16, kind="ExternalInput").ap()
    wbt = nc.dram_tensor("wbt", [C, C], bf16, kind="ExternalInput").ap()
    pwt = nc.dram_tensor("pwt", [N_TCN, C, C], bf16, kind="ExternalInput").ap()
    dwdiag = nc.dram_tensor(
        "dwdiag", [N_TCN, 2, KC, 128, 128], bf16, kind="ExternalInput"
    ).ap()
    dwc = nc.dram_tensor("dwc", [128, N_TCN, KC, 3], f32, kind="ExternalInput").ap()
    gbf = nc.dram_tensor("gbf", [128, N_TCN, KC, 2], f32, kind="ExternalInput").ap()
    encgb = nc.dram_tensor("encgb", [128, KC, 2], f32, kind="ExternalInput").ap()
    alph = nc.dram_tensor("alph", [1, 26], f32, kind="ExternalInput").ap()
    w1t = nc.dram_tensor("w1t", [C, 256], bf16, kind="ExternalInput").ap()
    w2t = nc.dram_tensor("w2t", [256, 1], bf16, kind="ExternalInput").ap()
    b1c = nc.dram_tensor("b1c", [128, 2], f32, kind="ExternalInput").ap()
    out = nc.dram_tensor("out", [1, SPC], f32, kind="ExternalOutput").ap()
    if K_DEBUG:
        dbg_f = nc.dram_tensor("dbg_f", [KC, 128, FW], bf16, kind="ExternalOutput").ap()
        dbg_g = nc.dram_tensor("dbg_g", [KC, 128, TE], bf16, kind="ExternalOutput").ap()
        dbg_h = nc.dram_tensor("dbg_h", [128, KC, SPC], bf16, kind="ExternalOutput").ap()
        dbg_pre = nc.dram_tensor("dbg_pre", [1, SPC], f32, kind="ExternalOutput").ap()

    ident_d = nc.inline_tensor(np.eye(128, dtype=np.float32), name="ident").ap()

    with tile.TileContext(nc) as tc:
        import contextlib

        ctx = contextlib.ExitStack()
        with ctx:
            const = ctx.enter_context(tc.tile_pool(name="const", bufs=1))
            poolf = ctx.enter_context(tc.tile_pool(name="poolf", bufs=1))
            poolg = ctx.enter_context(tc.tile_pool(name="poolg", bufs=1))
            poolw = ctx.enter_context(tc.tile_pool(name="poolw", bufs=3))
            poolscr = ctx.enter_context(tc.tile_pool(name="poolscr", bufs=3))
            poolst = ctx.enter_context(tc.tile_pool(name="poolst", bufs=3))
            poolps = ctx.enter_context(tc.tile_pool(name="poolps", bufs=2, space="PSUM"))
            poolps2 = ctx.enter_context(tc.tile_pool(name="poolps2", bufs=2, space="PSUM"))

            # ---- constants / weights resident in SBUF ----
            alpha_sb = const.tile([128, 26], f32, name="alpha_sb")
            nc.sync.dma_start(
                out=alpha_sb,
                in_=bass.AP(tensor=alph.tensor, offset=alph.offset, ap=[[0, 128], [1, 26]]),
            )
            dwc_sb = const.tile([128, N_TCN, KC, 3], f32, name="dwc_sb")
            nc.sync.dma_start(out=dwc_sb, in_=dwc)
            gbf_sb = const.tile([128, N_TCN, KC, 2], f32, name="gbf_sb")
            nc.sync.dma_start(out=gbf_sb, in_=gbf)
            encgb_sb = const.tile([128, KC, 2], f32, name="encgb_sb")
            nc.sync.dma_start(out=encgb_sb, in_=encgb)
            enca = const.tile([20, C], bf16, name="enca")
            nc.sync.dma_start(out=enca, in_=encw[0])
            encb = const.tile([20, C], bf16, name="encb")
            nc.sync.dma_start(out=encb, in_=encw[1])
            wbt_sb = const.tile([128, KC, C], bf16, name="wbt_sb")
            nc.sync.dma_start(out=wbt_sb, in_=wbt.rearrange("(kc kp) m -> kp kc m", kp=128))
            w1t_sb = const.tile([128, KC, 256], bf16, name="w1t_sb")
            nc.sync.dma_start(out=w1t_sb, in_=w1t.rearrange("(kc kp) m -> kp kc m", kp=128))
            w2t_sb = const.tile([128, 2], bf16, name="w2t_sb")
            nc.sync.dma_start(out=w2t_sb, in_=w2t.rearrange("(mt kp) o -> kp (mt o)", kp=128))
            b1c_sb = const.tile([128, 2], f32, name="b1c_sb")
            nc.sync.dma_start(out=b1c_sb, in_=b1c)
            ident = const.tile([128, 128], f32, name="ident")
            nc.sync.dma_start(out=ident, in_=ident_d)
            identb = const.tile([128, 128], bf16, name="identb")
            nc.vector.tensor_copy(identb, ident)
            ones128 = const.tile([128, 1], f32, name="ones128")
            nc.vector.memset(ones128, 1.0)
            ones_r = const.tile([1, 128], f32, name="ones_r")
            nc.vector.memset(ones_r, 1.0)
            eps_sb = const.tile([128, 1], f32, name="eps_sb")
            nc.vector.memset(eps_sb, EPS)

            # ---- persistent activation state ----
            f = [
                [poolf.tile([128, FW], bf16, name=f"f_{s}_{k}", tag=f"f_{s}_{k}") for k in range(KC)]
                for s in range(SPC)
            ]
            g = [
                [poolg.tile([128, TE], bf16, name=f"g_{s}_{k}", tag=f"g_{s}_{k}") for k in range(KC)]
                for s in range(SPC)
            ]
            for s in range(SPC):
                for k in range(KC):
                    nc.gpsimd.memset(f[s][k], 0.0)
                    nc.gpsimd.memset(g[s][k], 0.0)

            # pooled-sum accumulator slots: [bnti, layer0..23] x 3 spans
            hacc = [
                poolst.tile(
                    [128, KC, N_TCN + 1, 3], f32, name=f"hacc_{s}", tag=f"hacc{s}", bufs=1
                )
                for s in range(SPC)
            ]

            if SKIP_ENC:
                for s in range(SPC):
                    nc.vector.memset(hacc[s], 0.0)

            # ---------------- helpers ----------------
            def stat_chain(s, S_ap, Q_ap, gb0, gb1, alpha_idx):
                """Emit scalar chain: rs = 1/sqrt(var+eps) (+1 Newton step) and
                the (128,KC) scale/bias tiles for the fused GN+PReLU pass."""
                mt_ = poolst.tile([1, 1], f32, name=f"mt_{s}", tag=f"mt{s}")
                nc.vector.tensor_scalar(out=mt_, in0=S_ap, scalar1=1.0 / CT, scalar2=None, op0=OP.mult)
                negmsq = poolst.tile([1, 1], f32, name=f"negmsq_{s}", tag=f"negmsq{s}")
                nc.vector.tensor_scalar(
                    out=negmsq, in0=mt_, scalar1=mt_, scalar2=-1.0, op0=OP.mult, op1=OP.mult
                )
                vt = poolst.tile([1, 1], f32, name=f"vt_{s}", tag=f"vt{s}")
                nc.vector.scalar_tensor_tensor(
                    out=vt, in0=Q_ap, scalar=1.0 / CT, in1=negmsq, op0=OP.mult, op1=OP.add
                )
                std = poolst.tile([1, 1], f32, name=f"std_{s}", tag=f"std{s}")
                nc.scalar.activation(out=std, in_=vt, func=AF.Sqrt, bias=eps_sb[:1], scale=1.0)
                r0 = poolst.tile([1, 1], f32, name=f"r0_{s}", tag=f"r0{s}")
                nc.vector.reciprocal(out=r0, in_=std)
                tn = poolst.tile([1, 1], f32, name=f"tn_{s}", tag=f"tn{s}")
                nc.vector.tensor_mul(tn, std, r0)
                nc.vector.tensor_scalar(out=tn, in0=tn, scalar1=-1.0, scalar2=2.0, op0=OP.mult, op1=OP.add)
                rsn = poolst.tile([1, 2], f32, name=f"rsn_{s}", tag=f"rsn{s}")
                nc.vector.tensor_mul(rsn[:, 0:1], r0, tn)
                nc.vector.tensor_scalar(
                    out=rsn[:, 1:2], in0=mt_, scalar1=rsn[:, 0:1], scalar2=-1.0, op0=OP.mult, op1=OP.mult
                )
                ps_bc = poolps2.tile([128, 2], f32, name=f"psbc_{s}", tag="misc")
                nc.tensor.matmul(ps_bc, ones_r, rsn, start=True, stop=True)
                bc = poolst.tile([128, 2], f32, name=f"bc_{s}", tag=f"bc{s}")
                nc.vector.tensor_copy(bc, ps_bc)
                sc_ = poolst.tile([128, KC], f32, name=f"sc_{s}", tag=f"sc{s}")
                nc.vector.tensor_scalar(out=sc_, in0=gb0, scalar1=bc[:, 0:1], scalar2=None, op0=OP.mult)
                sb_ = poolst.tile([128, KC], f32, name=f"sb_{s}", tag=f"sb{s}")
                nc.vector.scalar_tensor_tensor(
                    out=sb_, in0=gb0, scalar=bc[:, 1:2], in1=gb1, op0=OP.mult, op1=OP.add
                )
                return sc_, sb_, alpha_sb[:, alpha_idx : alpha_idx + 1]

            def prelu_pass(s, sc_, sb_, alpha_ap):
                for k in range(KC):
                    nc.scalar.activation(
                        out=g[s][k][:, :T],
                        in_=g[s][k][:, :T],
                        func=AF.Prelu,
                        bias=sb_[:, k : k + 1],
                        scale=sc_[:, k : k + 1],
                        alpha=alpha_ap,
                    )

            # ================= encoder =================
            rhs20 = []
            if not SKIP_ENC:
              with tc.tile_pool(name="poolenc", bufs=1) as poolenc, tc.tile_pool(
                name="poolxc", bufs=3
              ) as poolxc:
                for s in range(SPC):
                    r = poolenc.tile([20, TE], bf16, name=f"rhs20_{s}", tag=f"rhs20_{s}")
                    rhs20.append(r)
                    nc.gpsimd.memset(r, 0.0)
                    for jg in range(7):  # groups of 4 chunks of 128 windows
                        ps_t = poolps2.tile([20, 512], f32, name=f"ps_t_{s}_{jg}", tag="misc")
                        width = 0
                        for jj in range(4):
                            j = jg * 4 + jj
                            if j >= 25:
                                break
                            nwin = min(128, 3200 - j * 128)
                            xc = poolxc.tile([128, 20], f32, name=f"xc_{s}_{j}", tag="xc")
                            nc.sync.dma_start(
                                out=xc[:nwin],
                                in_=x2[s, j * 2560 : j * 2560 + nwin * 20].rearrange(
                                    "(r k) -> r k", k=20
                                ),
                            )
                            nc.tensor.transpose(ps_t[:, jj * 128 : jj * 128 + nwin], xc[:nwin], ident)
                            width += nwin
                        nc.scalar.copy(
                            out=r[:, 1 + jg * 512 : 1 + jg * 512 + width], in_=ps_t[:, :width]
                        )

                eacc = [
                    poolst.tile([128, 32], f32, name=f"eacc_{s}", tag=f"eacc{s}", bufs=1)
                    for s in range(SPC)
                ]
                for s in range(SPC):
                    for m in range(KC):
                        for nt, (n0, w) in enumerate(NT_SPANS):
                            ps = poolps.tile([128, PW], f32, name=f"ps_e_{s}_{m}_{nt}", tag="mm")
                            for sub in range(0, w, 512):
                                nn = min(512, w - sub)
                                o = n0 + sub
                                nc.tensor.matmul(
                                    ps[:, sub : sub + nn],
                                    enca[:, m * 128 : (m + 1) * 128],
                                    rhs20[s][:, o : o + nn],
                                    start=True,
                                    stop=False,
                                )
                                nc.tensor.matmul(
                                    ps[:, sub : sub + nn],
                                    encb[:, m * 128 : (m + 1) * 128],
                                    rhs20[s][:, o + 1 : o + 1 + nn],
                                    start=False,
                                    stop=True,
                                )
                                nc.vector.tensor_scalar(
                                    out=g[s][m][:, o : o + nn],
                                    in0=ps[:, sub : sub + nn],
                                    scalar1=1.0,
                                    scalar2=None,
                                    op0=OP.mult,
                                    op1=OP.add,
                                    accum_out=eacc[s][:, m * 7 + (n0 + sub) // 512 : m * 7 + (n0 + sub) // 512 + 1],
                                )
                    for m in range(KC):
                        scr = poolscr.tile([128, TE], bf16, name=f"scr_e_{s}_{m}", tag="gsq")
                        nc.vector.scalar_tensor_tensor(
                            out=scr[:, :T],
                            in0=g[s][m][:, :T],
                            scalar=1.0,
                            in1=g[s][m][:, :T],
                            op0=OP.mult,
                            op1=OP.mult,
                            accum_out=eacc[s][:, 28 + m : 29 + m],
                        )

                for s in range(SPC):
                    ps_st = poolps2.tile([1, 32], f32, name=f"ps_ste_{s}", tag="misc")
                    nc.tensor.matmul(ps_st, ones128, eacc[s], start=True, stop=True)
                    SQ = poolst.tile([1, 2], f32, name=f"SQe_{s}", tag=f"SQ{s}")
                    nc.vector.tensor_reduce(
                        out=SQ[:, 0:1], in_=ps_st[:, 0:28], axis=AX.X, op=OP.add
                    )
                    nc.vector.tensor_reduce(
                        out=SQ[:, 1:2], in_=ps_st[:, 28:32], axis=AX.X, op=OP.add
                    )
                    sc_, sb_, a_ = stat_chain(
                        s, SQ[:, 0:1], SQ[:, 1:2], encgb_sb[:, :, 0], encgb_sb[:, :, 1], 0
                    )
                    prelu_pass(s, sc_, sb_, a_)

                # fused bottleneck+tcn_input 1x1 (integer matrix) -> f0
                for s in range(SPC):
                    for m in range(KC):
                        for nt, (n0, w) in enumerate(NT_SPANS):
                            ps = poolps.tile([128, PW], f32, name=f"ps_bt_{s}_{m}_{nt}", tag="mm")
                            for sub in range(0, w, 512):
                                nn = min(512, w - sub)
                                o = n0 + sub
                                for kc in range(KC):
                                    nc.tensor.matmul(
                                        ps[:, sub : sub + nn],
                                        wbt_sb[:, kc, m * 128 : (m + 1) * 128],
                                        g[s][kc][:, o : o + nn],
                                        start=(kc == 0),
                                        stop=(kc == 3),
                                    )
                            nc.vector.tensor_scalar(
                                out=f[s][m][:, PAD + n0 : PAD + n0 + w],
                                in0=ps[:, :w],
                                scalar1=cbt,
                                scalar2=None,
                                op0=OP.mult,
                                op1=OP.add,
                                accum_out=hacc[s][:, m, 0, nt : nt + 1],
                            )

            # ================= TCN layers =================
            for i in range(N_LAYERS):
                d = DILATIONS[i]
                wtile = poolw.tile([128, KC, C], bf16, name=f"wt_{i}", tag="pw")
                nc.sync.dma_start(out=wtile, in_=pwt[i].rearrange("(kc kp) m -> kp kc m", kp=128))
                dtile = poolw.tile([128, 2, KC, 128], bf16, name=f"dt_{i}", tag="dw")
                nc.sync.dma_start(
                    out=dtile, in_=dwdiag[i].rearrange("j k p m -> p j k m")
                )
                for s in range(SPC):
                    acc = poolst.tile([128, 8], f32, name=f"acc_{i}_{s}", tag=f"acc{s}")
                    # ---- sum(g) via edge-corrected residual sums (no pass) ----
                    S_t = poolst.tile([128, KC], f32, name=f"S_{i}_{s}", tag=f"S{s}")
                    nc.vector.tensor_reduce(
                        out=S_t, in_=hacc[s][:, :, i, :], axis=AX.X, op=OP.add
                    )
                    ee = poolst.tile([128, 2, KC], f32, name=f"ee_{i}_{s}", tag=f"ee{s}")
                    for k in range(KC):
                        nc.vector.tensor_reduce(
                            out=ee[:, 0, k : k + 1], in_=f[s][k][:, PAD : PAD + d],
                            axis=AX.X, op=OP.add,
                        )
                        nc.vector.tensor_reduce(
                            out=ee[:, 1, k : k + 1], in_=f[s][k][:, PAD + T - d : PAD + T],
                            axis=AX.X, op=OP.add,
                        )
                    w0c = dwc_sb[:, i, :, 0]
                    w1c = dwc_sb[:, i, :, 1]
                    w2c = dwc_sb[:, i, :, 2]
                    t1 = poolst.tile([128, KC], f32, name=f"t1_{i}_{s}", tag=f"t1{s}")
                    nc.vector.tensor_sub(t1, S_t, ee[:, 1, :])   # S - Er
                    nc.vector.tensor_mul(t1, t1, w0c)
                    t2 = poolst.tile([128, KC], f32, name=f"t2_{i}_{s}", tag=f"t2{s}")
                    nc.vector.tensor_sub(t2, S_t, ee[:, 0, :])   # S - El
                    nc.vector.tensor_mul(t2, t2, w2c)
                    nc.vector.tensor_add(t1, t1, t2)
                    t3 = poolst.tile([128, KC], f32, name=f"t3_{i}_{s}", tag=f"t3{s}")
                    nc.vector.tensor_mul(t3, S_t, w1c)
                    nc.vector.tensor_add(acc[:, 0:4], t1, t3)

                    # ---- depthwise conv ----
                    for k in range(KC):
                        for nt, (n0, w) in enumerate(NT_SPANS):
                            tp = poolps.tile([128, PW], f32, name=f"tp_{i}_{s}_{k}_{nt}", tag="mm")
                            for sub in range(0, w, 512):
                                nn = min(512, w - sub)
                                o = PAD + n0 + sub
                                nc.tensor.matmul(
                                    tp[:, sub : sub + nn],
                                    dtile[:, 0, k, :],
                                    f[s][k][:, o - d : o - d + nn],
                                    start=True,
                                    stop=False,
                                )
                                nc.tensor.matmul(
                                    tp[:, sub : sub + nn],
                                    dtile[:, 1, k, :],
                                    f[s][k][:, o + d : o + d + nn],
                                    start=False,
                                    stop=True,
                                )
                            # g = f_c * w1 + taps
                            nc.vector.scalar_tensor_tensor(
                                out=g[s][k][:, n0 : n0 + w],
                                in0=f[s][k][:, PAD + n0 : PAD + n0 + w],
                                scalar=w1c[:, k : k + 1],
                                in1=tp[:, :w],
                                op0=OP.mult,
                                op1=OP.add,
                            )
                    # ---- sum(g^2) ----
                    for k in range(KC):
                        scr = poolscr.tile([128, TE], bf16, name=f"scr_{i}_{s}_{k}", tag="gsq")
                        if K_SUMSQ == "act":
                            nc.scalar.activation(
                                out=scr[:, :T],
                                in_=g[s][k][:, :T],
                                func=AF.Square,
                                accum_out=acc[:, 4 + k : 5 + k],
                            )
                        else:
                            nc.vector.scalar_tensor_tensor(
                                out=scr[:, :T],
                                in0=g[s][k][:, :T],
                                scalar=1.0,
                                in1=g[s][k][:, :T],
                                op0=OP.mult,
                                op1=OP.mult,
                                accum_out=acc[:, 4 + k : 5 + k],
                            )
                    # ---- GN stats + PReLU ----
                    ps_st = poolps2.tile([1, 8], f32, name=f"ps_st_{i}_{s}", tag="misc")
                    nc.tensor.matmul(ps_st, ones128, acc, start=True, stop=True)
                    SQ = poolst.tile([1, 2], f32, name=f"SQ_{i}_{s}", tag=f"SQ{s}")
                    nc.vector.tensor_reduce(
                        out=SQ,
                        in_=ps_st.rearrange("p (a b) -> p a b", a=2),
                        axis=AX.X,
                        op=OP.add,
                    )
                    sc_, sb_, a_ = stat_chain(
                        s, SQ[:, 0:1], SQ[:, 1:2], gbf_sb[:, i, :, 0], gbf_sb[:, i, :, 1], 1 + i
                    )
                    prelu_pass(s, sc_, sb_, a_)
                    # ---- pointwise 1x1 + residual (+ pooled-sum accum) ----
                    for m in range(KC):
                        for nt, (n0, w) in enumerate(NT_SPANS):
                            ps = poolps.tile([128, PW], f32, name=f"ps_{i}_{s}_{m}_{nt}", tag="mm")
                            for sub in range(0, w, 512):
                                nn = min(512, w - sub)
                                o = n0 + sub
                                for kc in range(KC):
                                    nc.tensor.matmul(
                                        ps[:, sub : sub + nn],
                                        wtile[:, kc, m * 128 : (m + 1) * 128],
                                        g[s][kc][:, o : o + nn],
                                        start=(kc == 0),
                                        stop=(kc == 3 and K_RESID != "act"),
                                    )
                                if K_RESID == "act":
                                    nc.tensor.matmul(
                                        ps[:, sub : sub + nn],
                                        identb,
                                        f[s][m][:, PAD + o : PAD + o + nn],
                                        start=False,
                                        stop=True,
                                    )
                            fslice = f[s][m][:, PAD + n0 : PAD + n0 + w]
                            if K_RESID == "act":
                                nc.scalar.activation(
                                    out=fslice,
                                    in_=ps[:, :w],
                                    func=AF.Copy,
                                    accum_out=hacc[s][:, m, i + 1, nt : nt + 1],
                                )
                            else:
                                nc.vector.scalar_tensor_tensor(
                                    out=fslice,
                                    in0=ps[:, :w],
                                    scalar=1.0,
                                    in1=fslice,
                                    op0=OP.mult,
                                    op1=OP.add,
                                    accum_out=hacc[s][:, m, i + 1, nt : nt + 1],
                                )

            # ================= debug dumps =================
            if K_DEBUG:
                for k in range(KC):
                    nc.sync.dma_start(out=dbg_f[k], in_=f[0][k])
                    nc.sync.dma_start(out=dbg_g[k], in_=g[0][k])

            # ================= head =================
            hcol = [const.tile([128, SPC], bf16, name=f"hcol_{k}") for k in range(KC)]
            habs2 = const.tile([128, SPC], f32, name="habs2")
            for s in range(SPC):
                hred = poolst.tile([128, KC], f32, name=f"hred_{s}", tag=f"hred{s}")
                nc.vector.tensor_reduce(
                    out=hred,
                    in_=hacc[s][:, :, N_LAYERS, :],
                    axis=AX.X,
                    op=OP.add,
                )
                for k in range(KC):
                    nc.vector.tensor_scalar(
                        out=hcol[k][:, s : s + 1],
                        in0=hred[:, k : k + 1],
                        scalar1=1.0 / T,
                        scalar2=None,
                        op0=OP.mult,
                    )
                nc.vector.tensor_reduce(
                    out=habs2[:, s : s + 1],
                    in_=hred,
                    axis=AX.X,
                    op=OP.add,
                    apply_absolute_value=True,
                )
            ps_x1 = poolps2.tile([1, SPC], f32, name="ps_x1", tag="misc")
            nc.tensor.matmul(ps_x1, ones128, habs2, start=True, stop=True)
            xs1 = const.tile([1, SPC], f32, name="xs1")
            nc.vector.tensor_scalar(
                out=xs1, in0=ps_x1, scalar1=1.0 / (C * T), scalar2=None, op0=OP.mult
            )
            ps_xb = poolps2.tile([128, SPC], f32, name="ps_xb", tag="misc")
            nc.tensor.matmul(ps_xb, ones_r, xs1, start=True, stop=True)
            xs1b = const.tile([128, SPC], f32, name="xs1b")
            nc.vector.tensor_copy(xs1b, ps_xb)

            h2 = [const.tile([128, SPC], bf16, name=f"h2_{mt}") for mt in range(2)]
            habs_h = [const.tile([128, SPC], f32, name=f"habs_h_{mt}") for mt in range(2)]
            ps_x2 = poolps2.tile([1, SPC], f32, name="ps_x2", tag="misc")
            for mt in range(2):
                ps_h = poolps2.tile([128, SPC], f32, name=f"ps_h_{mt}", tag="misc")
                for kc in range(KC):
                    nc.tensor.matmul(
                        ps_h,
                        w1t_sb[:, kc, mt * 128 : (mt + 1) * 128],
                        hcol[kc],
                        start=(kc == 0),
                        stop=(kc == 3),
                    )
                bt = const.tile([128, SPC], f32, name=f"bt_{mt}")
                nc.vector.tensor_scalar(
                    out=bt, in0=xs1b, scalar1=b1c_sb[:, mt : mt + 1], scalar2=None, op0=OP.mult
                )
                nc.vector.scalar_tensor_tensor(
                    out=h2[mt], in0=ps_h, scalar=c1, in1=bt, op0=OP.mult, op1=OP.add
                )
                nc.scalar.activation(
                    out=h2[mt], in_=h2[mt], func=AF.Prelu, bias=0.0, scale=1.0,
                    alpha=alpha_sb[:, 25:26],
                )
                nc.scalar.activation(out=habs_h[mt], in_=h2[mt], func=AF.Abs)
                nc.tensor.matmul(ps_x2, ones128, habs_h[mt], start=(mt == 0), stop=(mt == 1))
            xs2 = const.tile([1, SPC], f32, name="xs2")
            nc.vector.tensor_scalar(out=xs2, in0=ps_x2, scalar1=1.0 / 256, scalar2=None, op0=OP.mult)
            ps_o = poolps2.tile([1, SPC], f32, name="ps_o", tag="misc")
            for mt in range(2):
                nc.tensor.matmul(
                    ps_o, w2t_sb[:, mt : mt + 1], h2[mt], start=(mt == 0), stop=(mt == 1)
                )
            pre = const.tile([1, SPC], f32, name="pre")
            nc.vector.tensor_scalar(out=pre, in0=xs2, scalar1=b2c, scalar2=None, op0=OP.mult)
            pre2 = const.tile([1, SPC], f32, name="pre2")
            nc.vector.scalar_tensor_tensor(
                out=pre2, in0=ps_o, scalar=c2, in1=pre, op0=OP.mult, op1=OP.add
            )
            score = const.tile([1, SPC], f32, name="score")
            nc.scalar.activation(out=score, in_=pre2, func=AF.Sigmoid)
            nc.sync.dma_start(out=out, in_=score)
            if K_DEBUG:
                nc.sync.dma_start(out=dbg_pre, in_=pre2)
                for k in range(KC):
                    nc.sync.dma_start(out=dbg_h[:, k, :], in_=hcol[k])

    nc.compile()
    return nc


def _get_nc(baked):
    key = hashlib.sha256(
        (repr(baked) + f"|{N_LAYERS}|{SKIP_ENC}|{K_DEBUG}|{K_SUMSQ}|{K_RESID}").encode()
    ).hexdigest()
    if key not in _CACHE:
        _CACHE[key] = _build(baked)
    return _CACHE[key]


def _in_maps(x, arrays):
    maps = []
    for c in range(NCORES):
        m = dict(arrays)
        m["x2"] = _f32(x[c * SPC : (c + 1) * SPC, 0, :])
        maps.append(m)
    return maps


def _gather(res):
    scores = np.empty((B, 1), dtype=np.float32)
    for c in range(NCORES):
        o = res.results[c]["out"]  # (1, SPC)
        for s in range(SPC):
            scores[c * SPC + s, 0] = o[0, s]
    return scores


def kernel(**inputs) -> np.ndarray:
    from concourse.bass_utils import run_bass_kernel_spmd

    x, arrays, baked = _prep(inputs)
    nc = _get_nc(baked)
    res = run_bass_kernel_spmd(nc, _in_maps(x, arrays), core_ids=list(range(NCORES)))
    return _gather(res)


def run_profiled(inputs, tmpdir=None):
    import ntff_shim

    ntff_shim.install()
    from concourse.bass_utils import run_bass_kernel_spmd

    x, arrays, baked = _prep(inputs)
    nc = _get_nc(baked)
    res = run_bass_kernel_spmd(
        nc, _in_maps(x, arrays), core_ids=list(range(NCORES)), trace=True, tmpdir=tmpdir
    )
    return _gather(res), res


# revision 18
# speedup vs baseline: 1.6989x; 1.6989x over previous
"""Trainium2 Bass kernel for nn_APDIntelligibilityEstimator.

Model: audio encoder conv(k=40,s=20) -> GroupNorm(1)+PReLU -> two 1x1
BitConv (fused into one 512x512 int matmul on host) -> 24 depthwise-separable
TCN blocks (dconv k=3 dil 2^(i%8) -> GN+PReLU -> 1x1 BitConv -> residual)
-> global avg pool -> BitLinear -> PReLU -> BitLinear -> sigmoid.

Sharding: data-parallel over batch, 2 samples per core on 8 NeuronCores.

Host-side weight preprocessing exploits exact algebraic identities of the
reference (x_scale cancellation in bit ops; composition of the two 1x1 sign
convs into one integer matrix; folding of w_scale*scale into the GN
affine+PReLU pass).

Device mapping per TCN layer:
  - depthwise side taps (t-d, t+d) -> TensorE diagonal matmuls into PSUM
  - center tap + combine           -> DVE scalar_tensor_tensor from PSUM
  - sum(g) for GN mean             -> edge-corrected from residual accum_out
                                      (no full pass)
  - sum(g^2) for GN var            -> ScalarE Square pass with accum_out
  - GN affine + PReLU + w_scale    -> one ScalarE Prelu pass (per-channel
                                      scale/bias APs)
  - 1x1 conv (+-1 weights, bf16)   -> TensorE matmuls
  - residual add + pooled sums     -> DVE scalar_tensor_tensor with accum_out
"""

import hashlib
import os
import sys

import numpy as np

sys.path.insert(0, os.path.dirname(os.path.abspath(__file__)))

N_TCN = 24
DILATIONS = [2 ** (i % 8) for i in range(N_TCN)]
B, T_IN = 16, 64000
T = 3201           # conv output length
TE = 3202          # allocated elementwise width (col T stays zero)
PAD = 128          # max dilation; zero padding on both sides of f
FW = PAD + TE + PAD
C = 512
KC = 4             # channel chunks of 128
CT = C * T
EPS = 1e-5
NCORES = 8
SPC = 2            # samples per core
PW = 1536          # psum macro-tile width (3 banks)
NT_SPANS = [(0, 1536), (1536, 1536), (3072, 129)]

N_LAYERS = int(os.environ.get("K_NLAYERS", str(N_TCN)))  # debug knob
SKIP_ENC = os.environ.get("K_SKIPENC", "0") == "1"        # debug knob
K_DEBUG = os.environ.get("K_DEBUG", "0") == "1"           # debug knob
K_SUMSQ = os.environ.get("K_SUMSQ", "act")                # act | dve
K_RESID = os.environ.get("K_RESID", "dve")                # dve | act
K_PW = os.environ.get("K_PW", "bf16")                     # bf16 | fp8
ATE = 3216         # fp8 act inner width (16-byte aligned)


def _f32(a):
    return np.ascontiguousarray(a, dtype=np.float32)


def _bf16(a):
    import ml_dtypes

    return np.ascontiguousarray(np.asarray(a, dtype=np.float32).astype(ml_dtypes.bfloat16))


def _prep(inp):
    """Host-side weight preprocessing. Returns (x, arrays, baked_scalars)."""
    x = np.asarray(inp["x"], dtype=np.float32)          # (16,1,64000)
    enc_w = np.asarray(inp["enc_w"], dtype=np.float32)  # (512,1,40)
    ew = enc_w[:, 0, :]
    encw = np.stack([ew[:, :20].T, ew[:, 20:].T])        # (2,20,512)

    sign = np.sign
    bn_w = np.asarray(inp["bn_w"], dtype=np.float32)[:, :, 0]    # (256,512)
    ti_w = np.asarray(inp["ti_w"], dtype=np.float32)[:, :, 0]    # (512,256)
    wbt = (sign(ti_w) @ sign(bn_w)).T                            # (512,512) ints
    cbt = float(
        np.abs(bn_w).mean() * inp["bn_scale"][0] * np.abs(ti_w).mean() * inp["ti_scale"][0]
    )

    pw_w = np.asarray(inp["pw_w"], dtype=np.float32)[:, :, :, 0]  # (24,512,512)
    c_pw = np.abs(pw_w).mean(axis=(1, 2)) * np.asarray(inp["pw_scale"], dtype=np.float32)[:, 0]
    assert np.all(c_pw > 0)
    if K_PW == "fp8":
        # exact +-1 weights in fp8; c_pw applied at the residual op
        pwt = sign(pw_w).transpose(0, 2, 1)                       # (24,512,512) [in,out]
    else:
        # fold w_scale*scale into the +-1 weights (+-c, exact-ish in bf16)
        pwt = (sign(pw_w) * c_pw[:, None, None]).transpose(0, 2, 1)

    dw_w = np.asarray(inp["dw_w"], dtype=np.float32)[:, :, 0, :]  # (24,512,3)
    dwc = dw_w.reshape(24, KC, 128, 3).transpose(2, 0, 1, 3)      # (128,24,4,3)
    # diagonal weight matrices for side taps on TensorE: (24, 2, KC, 128, 128)
    dwdiag = np.zeros((N_TCN, 2, KC, 128, 128), dtype=np.float32)
    idx = np.arange(128)
    for i in range(N_TCN):
        for jt, tap in enumerate((0, 2)):
            for k in range(KC):
                dwdiag[i, jt, k, idx, idx] = dw_w[i, k * 128 : (k + 1) * 128, tap]

    tg = np.asarray(inp["tcn_gamma"], dtype=np.float32)  # (24,512)
    tb = np.asarray(inp["tcn_beta"], dtype=np.float32)
    gbf = np.stack(
        [
            tg.reshape(24, KC, 128).transpose(2, 0, 1),
            tb.reshape(24, KC, 128).transpose(2, 0, 1),
        ],
        axis=-1,
    )  # (128,24,4,2)

    encgb = np.stack(
        [
            np.asarray(inp["enc_gamma"], dtype=np.float32).reshape(KC, 128).T,
            np.asarray(inp["enc_beta"], dtype=np.float32).reshape(KC, 128).T,
        ],
        axis=-1,
    )  # (128,4,2)

    alph = np.concatenate(
        [
            np.atleast_1d(np.float32(inp["enc_prelu"])),
            np.asarray(inp["tcn_prelu"], dtype=np.float32),
            np.atleast_1d(np.float32(inp["head_prelu"])),
        ]
    )[None, :]  # (1,26)

    fc1_w = np.asarray(inp["fc1_w"], dtype=np.float32)   # (256,512)
    c1 = float(np.abs(fc1_w).mean() * inp["fc1_scale"][0])
    w1t = sign(fc1_w).T                                   # (512,256)
    b1c = (np.asarray(inp["fc1_b"], dtype=np.float32) * c1).reshape(2, 128).T  # (128,2)

    fco_w = np.asarray(inp["fco_w"], dtype=np.float32)   # (1,256)
    c2 = float(np.abs(fco_w).mean() * inp["fco_scale"][0])
    w2t = sign(fco_w).T                                   # (256,1)
    b2c = float(np.asarray(inp["fco_b"], dtype=np.float32)[0] * c2)

    def _fp8(a):
        import ml_dtypes
        return np.ascontiguousarray(np.asarray(a, dtype=np.float32).astype(ml_dtypes.float8_e4m3))

    arrays = dict(
        encw=_bf16(encw),
        wbt=_bf16(wbt),
        pwt=_fp8(pwt) if K_PW == "fp8" else _bf16(pwt),
        dwdiag=_bf16(dwdiag),
        dwc=_f32(dwc),
        gbf=_f32(gbf),
        encgb=_f32(encgb),
        alph=_f32(alph),
        w1t=_bf16(w1t),
        w2t=_bf16(w2t),
        b1c=_f32(b1c),
    )
    baked = (cbt, c1, b2c, float(c2), tuple(float(v) for v in c_pw))
    return x, arrays, baked


# ---------------------------------------------------------------------------

_CACHE = {}


def _build(baked):
    assert not (K_PW == "fp8" and K_RESID == "act")
    import concourse.bacc as bacc
    import concourse.bass as bass
    import concourse.mybir as mybir
    import concourse.tile as tile

    cbt, c1, b2c, c2, c_pw = baked
    f32 = mybir.dt.float32
    bf16 = mybir.dt.bfloat16
    AF = mybir.ActivationFunctionType
    OP = mybir.AluOpType
    AX = mybir.AxisListType

    nc = bacc.Bacc("TRN2", target_bir_lowering=False, debug=False)

    x2 = nc.dram_tensor("x2", [SPC, T_IN], f32, kind="ExternalInput").ap()
    encw = nc.dram_tensor("encw", [2, 20, C], bf16, kind="ExternalInput").ap()
    wbt = nc.dram_tensor("wbt", [C, C], bf16, kind="ExternalInput").ap()
    fp8 = mybir.dt.float8e4
    pw_dt = fp8 if K_PW == "fp8" else bf16
    pwt = nc.dram_tensor("pwt", [N_TCN, C, C], pw_dt, kind="ExternalInput").ap()
    dwdiag = nc.dram_tensor(
        "dwdiag", [N_TCN, 2, KC, 128, 128], bf16, kind="ExternalInput"
    ).ap()
    dwc = nc.dram_tensor("dwc", [128, N_TCN, KC, 3], f32, kind="ExternalInput").ap()
    gbf = nc.dram_tensor("gbf", [128, N_TCN, KC, 2], f32, kind="ExternalInput").ap()
    encgb = nc.dram_tensor("encgb", [128, KC, 2], f32, kind="ExternalInput").ap()
    alph = nc.dram_tensor("alph", [1, 26], f32, kind="ExternalInput").ap()
    w1t = nc.dram_tensor("w1t", [C, 256], bf16, kind="ExternalInput").ap()
    w2t = nc.dram_tensor("w2t", [256, 1], bf16, kind="ExternalInput").ap()
    b1c = nc.dram_tensor("b1c", [128, 2], f32, kind="ExternalInput").ap()
    out = nc.dram_tensor("out", [1, SPC], f32, kind="ExternalOutput").ap()
    if K_DEBUG:
        dbg_f = nc.dram_tensor("dbg_f", [KC, 128, FW], bf16, kind="ExternalOutput").ap()
        dbg_g = nc.dram_tensor("dbg_g", [KC, 128, TE], bf16, kind="ExternalOutput").ap()
        dbg_h = nc.dram_tensor("dbg_h", [128, KC, SPC], bf16, kind="ExternalOutput").ap()
        dbg_pre = nc.dram_tensor("dbg_pre", [1, SPC], f32, kind="ExternalOutput").ap()

    ident_d = nc.inline_tensor(np.eye(128, dtype=np.float32), name="ident").ap()

    with tile.TileContext(nc) as tc:
        import contextlib

        ctx = contextlib.ExitStack()
        with ctx:
            const = ctx.enter_context(tc.tile_pool(name="const", bufs=1))
            poolf = ctx.enter_context(tc.tile_pool(name="poolf", bufs=1))
            poolg = ctx.enter_context(tc.tile_pool(name="poolg", bufs=1))
            poolw = ctx.enter_context(tc.tile_pool(name="poolw", bufs=3))
            poolscr = ctx.enter_context(tc.tile_pool(name="poolscr", bufs=3))
            poolst = ctx.enter_context(tc.tile_pool(name="poolst", bufs=3))
            poolps = ctx.enter_context(tc.tile_pool(name="poolps", bufs=2, space="PSUM"))
            poolps2 = ctx.enter_context(tc.tile_pool(name="poolps2", bufs=2, space="PSUM"))

            # ---- constants / weights resident in SBUF ----
            alpha_sb = const.tile([128, 26], f32, name="alpha_sb")
            nc.sync.dma_start(
                out=alpha_sb,
                in_=bass.AP(tensor=alph.tensor, offset=alph.offset, ap=[[0, 128], [1, 26]]),
            )
            dwc_sb = const.tile([128, N_TCN, KC, 3], f32, name="dwc_sb")
            nc.sync.dma_start(out=dwc_sb, in_=dwc)
            gbf_sb = const.tile([128, N_TCN, KC, 2], f32, name="gbf_sb")
            nc.sync.dma_start(out=gbf_sb, in_=gbf)
            encgb_sb = const.tile([128, KC, 2], f32, name="encgb_sb")
            nc.sync.dma_start(out=encgb_sb, in_=encgb)
            enca = const.tile([20, C], bf16, name="enca")
            nc.sync.dma_start(out=enca, in_=encw[0])
            encb = const.tile([20, C], bf16, name="encb")
            nc.sync.dma_start(out=encb, in_=encw[1])
            wbt_sb = const.tile([128, KC, C], bf16, name="wbt_sb")
            nc.sync.dma_start(out=wbt_sb, in_=wbt.rearrange("(kc kp) m -> kp kc m", kp=128))
            w1t_sb = const.tile([128, KC, 256], bf16, name="w1t_sb")
            nc.sync.dma_start(out=w1t_sb, in_=w1t.rearrange("(kc kp) m -> kp kc m", kp=128))
            w2t_sb = const.tile([128, 2], bf16, name="w2t_sb")
            nc.sync.dma_start(out=w2t_sb, in_=w2t.rearrange("(mt kp) o -> kp (mt o)", kp=128))
            b1c_sb = const.tile([128, 2], f32, name="b1c_sb")
            nc.sync.dma_start(out=b1c_sb, in_=b1c)
            ident = const.tile([128, 128], f32, name="ident")
            nc.sync.dma_start(out=ident, in_=ident_d)
            identb = const.tile([128, 128], bf16, name="identb")
            nc.vector.tensor_copy(identb, ident)
            ones128 = const.tile([128, 1], f32, name="ones128")
            nc.vector.memset(ones128, 1.0)
            ones_r = const.tile([1, 128], f32, name="ones_r")
            nc.vector.memset(ones_r, 1.0)
            eps_sb = const.tile([128, 1], f32, name="eps_sb")
            nc.vector.memset(eps_sb, EPS)

            # ---- persistent activation state ----
            f = [
                [poolf.tile([128, FW], bf16, name=f"f_{s}_{k}", tag=f"f_{s}_{k}") for k in range(KC)]
                for s in range(SPC)
            ]
            g = [
                poolg.tile([128, KC, TE], bf16, name=f"g_{s}", tag=f"g_{s}")
                for s in range(SPC)
            ]
            act8 = None
            if K_PW == "fp8":
                act8 = [
                    poolg.tile([128, KC, ATE], fp8, name=f"act8_{s}", tag=f"act8_{s}")
                    for s in range(SPC)
                ]
            for s in range(SPC):
                for k in range(KC):
                    nc.gpsimd.memset(f[s][k], 0.0)
                nc.gpsimd.memset(g[s], 0.0)

            # pooled-sum accumulator slots: [bnti, layer0..23] x 3 spans
            hacc = [
                poolst.tile(
                    [128, KC, N_TCN + 1, 3], f32, name=f"hacc_{s}", tag=f"hacc{s}", bufs=1
                )
                for s in range(SPC)
            ]

            if SKIP_ENC:
                for s in range(SPC):
                    nc.vector.memset(hacc[s], 0.0)

            # ---------------- helpers ----------------
            def stat_chain(s, S_ap, Q_ap, gb0, gb1, alpha_idx):
                """Emit scalar chain: rs = 1/sqrt(var+eps) (+1 Newton step) and
                the (128,KC) scale/bias tiles for the fused GN+PReLU pass."""
                mt_ = poolst.tile([1, 1], f32, name=f"mt_{s}", tag=f"mt{s}")
                nc.vector.tensor_scalar(out=mt_, in0=S_ap, scalar1=1.0 / CT, scalar2=None, op0=OP.mult)
                negmsq = poolst.tile([1, 1], f32, name=f"negmsq_{s}", tag=f"negmsq{s}")
                nc.vector.tensor_scalar(
                    out=negmsq, in0=mt_, scalar1=mt_, scalar2=-1.0, op0=OP.mult, op1=OP.mult
                )
                vt = poolst.tile([1, 1], f32, name=f"vt_{s}", tag=f"vt{s}")
                nc.vector.scalar_tensor_tensor(
                    out=vt, in0=Q_ap, scalar=1.0 / CT, in1=negmsq, op0=OP.mult, op1=OP.add
                )
                std = poolst.tile([1, 1], f32, name=f"std_{s}", tag=f"std{s}")
                nc.scalar.activation(out=std, in_=vt, func=AF.Sqrt, bias=eps_sb[:1], scale=1.0)
                r0 = poolst.tile([1, 1], f32, name=f"r0_{s}", tag=f"r0{s}")
                nc.vector.reciprocal(out=r0, in_=std)
                tn = poolst.tile([1, 1], f32, name=f"tn_{s}", tag=f"tn{s}")
                nc.vector.tensor_mul(tn, std, r0)
                nc.vector.tensor_scalar(out=tn, in0=tn, scalar1=-1.0, scalar2=2.0, op0=OP.mult, op1=OP.add)
                rsn = poolst.tile([1, 2], f32, name=f"rsn_{s}", tag=f"rsn{s}")
                nc.vector.tensor_mul(rsn[:, 0:1], r0, tn)
                nc.vector.tensor_scalar(
                    out=rsn[:, 1:2], in0=mt_, scalar1=rsn[:, 0:1], scalar2=-1.0, op0=OP.mult, op1=OP.mult
                )
                ps_bc = poolps2.tile([128, 2], f32, name=f"psbc_{s}", tag="misc")
                nc.tensor.matmul(ps_bc, ones_r, rsn, start=True, stop=True)
                bc = poolst.tile([128, 2], f32, name=f"bc_{s}", tag=f"bc{s}")
                nc.vector.tensor_copy(bc, ps_bc)
                sc_ = poolst.tile([128, KC], f32, name=f"sc_{s}", tag=f"sc{s}")
                nc.vector.tensor_scalar(out=sc_, in0=gb0, scalar1=bc[:, 0:1], scalar2=None, op0=OP.mult)
                sb_ = poolst.tile([128, KC], f32, name=f"sb_{s}", tag=f"sb{s}")
                nc.vector.scalar_tensor_tensor(
                    out=sb_, in0=gb0, scalar=bc[:, 1:2], in1=gb1, op0=OP.mult, op1=OP.add
                )
                return sc_, sb_, alpha_sb[:, alpha_idx : alpha_idx + 1]

            def prelu_pass(s, sc_, sb_, alpha_ap, to_act8=False):
                for k in range(KC):
                    dst = act8[s][:, k, :T] if to_act8 else g[s][:, k, :T]
                    nc.scalar.activation(
                        out=dst,
                        in_=g[s][:, k, :T],
                        func=AF.Prelu,
                        bias=sb_[:, k : k + 1],
                        scale=sc_[:, k : k + 1],
                        alpha=alpha_ap,
                    )

            # ================= encoder =================
            rhs20 = []
            if not SKIP_ENC:
              with tc.tile_pool(name="poolenc", bufs=1) as poolenc, tc.tile_pool(
                name="poolxc", bufs=3
              ) as poolxc:
                for s in range(SPC):
                    r = poolenc.tile([20, TE], bf16, name=f"rhs20_{s}", tag=f"rhs20_{s}")
                    rhs20.append(r)
                    nc.gpsimd.memset(r, 0.0)
                    for jg in range(7):  # groups of 4 chunks of 128 windows
                        ps_t = poolps2.tile([20, 512], f32, name=f"ps_t_{s}_{jg}", tag="misc")
                        width = 0
                        for jj in range(4):
                            j = jg * 4 + jj
                            if j >= 25:
                                break
                            nwin = min(128, 3200 - j * 128)
                            xc = poolxc.tile([128, 20], f32, name=f"xc_{s}_{j}", tag="xc")
                            nc.sync.dma_start(
                                out=xc[:nwin],
                                in_=x2[s, j * 2560 : j * 2560 + nwin * 20].rearrange(
                                    "(r k) -> r k", k=20
                                ),
                            )
                            nc.tensor.transpose(ps_t[:, jj * 128 : jj * 128 + nwin], xc[:nwin], ident)
                            width += nwin
                        nc.scalar.copy(
                            out=r[:, 1 + jg * 512 : 1 + jg * 512 + width], in_=ps_t[:, :width]
                        )

                eacc = [
                    poolst.tile([128, 32], f32, name=f"eacc_{s}", tag=f"eacc{s}", bufs=1)
                    for s in range(SPC)
                ]
                for s in range(SPC):
                    for m in range(KC):
                        for nt, (n0, w) in enumerate(NT_SPANS):
                            ps = poolps.tile([128, PW], f32, name=f"ps_e_{s}_{m}_{nt}", tag="mm")
                            for sub in range(0, w, 512):
                                nn = min(512, w - sub)
                                o = n0 + sub
                                nc.tensor.matmul(
                                    ps[:, sub : sub + nn],
                                    enca[:, m * 128 : (m + 1) * 128],
                                    rhs20[s][:, o : o + nn],
                                    start=True,
                                    stop=False,
                                )
                                nc.tensor.matmul(
                                    ps[:, sub : sub + nn],
                                    encb[:, m * 128 : (m + 1) * 128],
                                    rhs20[s][:, o + 1 : o + 1 + nn],
                                    start=False,
                                    stop=True,
                                )
                                nc.vector.tensor_scalar(
                                    out=g[s][:, m, o : o + nn],
                                    in0=ps[:, sub : sub + nn],
                                    scalar1=1.0,
                                    scalar2=None,
                                    op0=OP.mult,
                                    op1=OP.add,
                                    accum_out=eacc[s][:, m * 7 + (n0 + sub) // 512 : m * 7 + (n0 + sub) // 512 + 1],
                                )
                    for m in range(KC):
                        scr = poolscr.tile([128, TE], bf16, name=f"scr_e_{s}_{m}", tag="gsq")
                        nc.vector.scalar_tensor_tensor(
                            out=scr[:, :T],
                            in0=g[s][:, m, :T],
                            scalar=1.0,
                            in1=g[s][:, m, :T],
                            op0=OP.mult,
                            op1=OP.mult,
                            accum_out=eacc[s][:, 28 + m : 29 + m],
                        )

                for s in range(SPC):
                    ps_st = poolps2.tile([1, 32], f32, name=f"ps_ste_{s}", tag="misc")
                    nc.tensor.matmul(ps_st, ones128, eacc[s], start=True, stop=True)
                    SQ = poolst.tile([1, 2], f32, name=f"SQe_{s}", tag=f"SQ{s}")
                    nc.vector.tensor_reduce(
                        out=SQ[:, 0:1], in_=ps_st[:, 0:28], axis=AX.X, op=OP.add
                    )
                    nc.vector.tensor_reduce(
                        out=SQ[:, 1:2], in_=ps_st[:, 28:32], axis=AX.X, op=OP.add
                    )
                    sc_, sb_, a_ = stat_chain(
                        s, SQ[:, 0:1], SQ[:, 1:2], encgb_sb[:, :, 0], encgb_sb[:, :, 1], 0
                    )
                    prelu_pass(s, sc_, sb_, a_)

                # fused bottleneck+tcn_input 1x1 (integer matrix) -> f0
                for s in range(SPC):
                    for m in range(KC):
                        for nt, (n0, w) in enumerate(NT_SPANS):
                            ps = poolps.tile([128, PW], f32, name=f"ps_bt_{s}_{m}_{nt}", tag="mm")
                            for sub in range(0, w, 512):
                                nn = min(512, w - sub)
                                o = n0 + sub
                                for kc in range(KC):
                                    nc.tensor.matmul(
                                        ps[:, sub : sub + nn],
                                        wbt_sb[:, kc, m * 128 : (m + 1) * 128],
                                        g[s][:, kc, o : o + nn],
                                        start=(kc == 0),
                                        stop=(kc == 3),
                                    )
                            nc.vector.tensor_scalar(
                                out=f[s][m][:, PAD + n0 : PAD + n0 + w],
                                in0=ps[:, :w],
                                scalar1=cbt,
                                scalar2=None,
                                op0=OP.mult,
                                op1=OP.add,
                                accum_out=hacc[s][:, m, 0, nt : nt + 1],
                            )

            # ================= TCN layers =================
            for i in range(N_LAYERS):
                d = DILATIONS[i]
                wtile = poolw.tile([128, KC, C], pw_dt, name=f"wt_{i}", tag="pw")
                nc.sync.dma_start(out=wtile, in_=pwt[i].rearrange("(kc kp) m -> kp kc m", kp=128))
                dtile = poolw.tile([128, 2, KC, 128], bf16, name=f"dt_{i}", tag="dw")
                nc.sync.dma_start(
                    out=dtile, in_=dwdiag[i].rearrange("j k p m -> p j k m")
                )
                w0c = dwc_sb[:, i, :, 0]
                w1c = dwc_sb[:, i, :, 1]
                w2c = dwc_sb[:, i, :, 2]
                accs = []
                # ---- sum(g) via edge-corrected residual sums (no pass) ----
                for s in range(SPC):
                    acc = poolst.tile([128, 8], f32, name=f"acc_{i}_{s}", tag=f"acc{s}")
                    accs.append(acc)
                    S_t = poolst.tile([128, KC], f32, name=f"S_{i}_{s}", tag=f"S{s}")
                    nc.vector.tensor_reduce(
                        out=S_t, in_=hacc[s][:, :, i, :], axis=AX.X, op=OP.add
                    )
                    ee = poolst.tile([128, 2, KC], f32, name=f"ee_{i}_{s}", tag=f"ee{s}")
                    for k in range(KC):
                        nc.vector.tensor_reduce(
                            out=ee[:, 0, k : k + 1], in_=f[s][k][:, PAD : PAD + d],
                            axis=AX.X, op=OP.add,
                        )
                        nc.vector.tensor_reduce(
                            out=ee[:, 1, k : k + 1], in_=f[s][k][:, PAD + T - d : PAD + T],
                            axis=AX.X, op=OP.add,
                        )
                    t1 = poolst.tile([128, KC], f32, name=f"t1_{i}_{s}", tag=f"t1{s}")
                    nc.vector.tensor_sub(t1, S_t, ee[:, 1, :])   # S - Er
                    nc.vector.tensor_mul(t1, t1, w0c)
                    t2 = poolst.tile([128, KC], f32, name=f"t2_{i}_{s}", tag=f"t2{s}")
                    nc.vector.tensor_sub(t2, S_t, ee[:, 0, :])   # S - El
                    nc.vector.tensor_mul(t2, t2, w2c)
                    nc.vector.tensor_add(t1, t1, t2)
                    t3 = poolst.tile([128, KC], f32, name=f"t3_{i}_{s}", tag=f"t3{s}")
                    nc.vector.tensor_mul(t3, S_t, w1c)
                    nc.vector.tensor_add(acc[:, 0:4], t1, t3)

                # ---- depthwise conv: side taps on PE, combine on DVE ----
                tps = {}
                for s in range(SPC):
                    for k in range(KC):
                        for nt, (n0, w) in enumerate(NT_SPANS):
                            tp = poolps.tile([128, PW], f32, name=f"tp_{i}_{s}_{k}_{nt}", tag="mm")
                            tps[(s, k, nt)] = tp
                            for sub in range(0, w, 512):
                                nn = min(512, w - sub)
                                o = PAD + n0 + sub
                                nc.tensor.matmul(
                                    tp[:, sub : sub + nn],
                                    dtile[:, 0, k, :],
                                    f[s][k][:, o - d : o - d + nn],
                                    start=True,
                                    stop=False,
                                )
                                nc.tensor.matmul(
                                    tp[:, sub : sub + nn],
                                    dtile[:, 1, k, :],
                                    f[s][k][:, o + d : o + d + nn],
                                    start=False,
                                    stop=True,
                                )
                            nc.vector.scalar_tensor_tensor(
                                out=g[s][:, k, n0 : n0 + w],
                                in0=f[s][k][:, PAD + n0 : PAD + n0 + w],
                                scalar=w1c[:, k : k + 1],
                                in1=tp[:, :w],
                                op0=OP.mult,
                                op1=OP.add,
                            )
                # ---- sum(g^2) ----
                for s in range(SPC):
                    for k in range(KC):
                        scr = poolscr.tile([128, TE], bf16, name=f"scr_{i}_{s}_{k}", tag="gsq")
                        if K_SUMSQ == "act":
                            nc.scalar.activation(
                                out=scr[:, :T],
                                in_=g[s][:, k, :T],
                                func=AF.Square,
                                accum_out=accs[s][:, 4 + k : 5 + k],
                            )
                        else:
                            nc.vector.scalar_tensor_tensor(
                                out=scr[:, :T],
                                in0=g[s][:, k, :T],
                                scalar=1.0,
                                in1=g[s][:, k, :T],
                                op0=OP.mult,
                                op1=OP.mult,
                                accum_out=accs[s][:, 4 + k : 5 + k],
                            )
                # ---- GN stats + PReLU ----
                for s in range(SPC):
                    ps_st = poolps2.tile([1, 8], f32, name=f"ps_st_{i}_{s}", tag="misc")
                    nc.tensor.matmul(ps_st, ones128, accs[s], start=True, stop=True)
                    SQ = poolst.tile([1, 2], f32, name=f"SQ_{i}_{s}", tag=f"SQ{s}")
                    nc.vector.tensor_reduce(
                        out=SQ,
                        in_=ps_st.rearrange("p (a b) -> p a b", a=2),
                        axis=AX.X,
                        op=OP.add,
                    )
                    sc_, sb_, a_ = stat_chain(
                        s, SQ[:, 0:1], SQ[:, 1:2], gbf_sb[:, i, :, 0], gbf_sb[:, i, :, 1], 1 + i
                    )
                    prelu_pass(s, sc_, sb_, a_, to_act8=(K_PW == "fp8"))
                # ---- pointwise 1x1 + residual (+ pooled-sum accum) ----
                for s in range(SPC):
                    for m in range(KC):
                        for nt, (n0, w) in enumerate(NT_SPANS):
                            ps = poolps.tile([128, PW], f32, name=f"ps_{i}_{s}_{m}_{nt}", tag="mm")
                            for sub in range(0, w, 512):
                                nn = min(512, w - sub)
                                o = n0 + sub
                                if K_PW == "fp8":
                                    for kc in (0, 2):
                                        nc.tensor.matmul(
                                            ps[:, sub : sub + nn],
                                            wtile[:, kc : kc + 2, m * 128 : (m + 1) * 128],
                                            act8[s][:, kc : kc + 2, o : o + nn],
                                            start=(kc == 0),
                                            stop=(kc == 2 and K_RESID != "act"),
                                            perf_mode=mybir.MatmulPerfMode.DoubleRow,
                                        )
                                else:
                                    for kc in range(KC):
                                        nc.tensor.matmul(
                                            ps[:, sub : sub + nn],
                                            wtile[:, kc, m * 128 : (m + 1) * 128],
                                            g[s][:, kc, o : o + nn],
                                            start=(kc == 0),
                                            stop=(kc == 3 and K_RESID != "act"),
                                        )
                                if K_RESID == "act":
                                    nc.tensor.matmul(
                                        ps[:, sub : sub + nn],
                                        identb,
                                        f[s][m][:, PAD + o : PAD + o + nn],
                                        start=False,
                                        stop=True,
                                    )
                            fslice = f[s][m][:, PAD + n0 : PAD + n0 + w]
                            rscale = c_pw[i] if K_PW == "fp8" else 1.0
                            if K_RESID == "act":
                                nc.scalar.activation(
                                    out=fslice,
                                    in_=ps[:, :w],
                                    func=AF.Copy,
                                    accum_out=hacc[s][:, m, i + 1, nt : nt + 1],
                                )
                            else:
                                nc.vector.scalar_tensor_tensor(
                                    out=fslice,
                                    in0=ps[:, :w],
                                    scalar=rscale,
                                    in1=fslice,
                                    op0=OP.mult,
                                    op1=OP.add,
                                    accum_out=hacc[s][:, m, i + 1, nt : nt + 1],
                                )

            # ================= debug dumps =================
            if K_DEBUG:
                for k in range(KC):
                    nc.sync.dma_start(out=dbg_f[k], in_=f[0][k])
                    nc.sync.dma_start(out=dbg_g[k], in_=g[0][:, k, :])

            # ================= head =================
            hcol = [const.tile([128, SPC], bf16, name=f"hcol_{k}") for k in range(KC)]
            habs2 = const.tile([128, SPC], f32, name="habs2")
            for s in range(SPC):
                hred = poolst.tile([128, KC], f32, name=f"hred_{s}", tag=f"hred{s}")
                nc.vector.tensor_reduce(
                    out=hred,
                    in_=hacc[s][:, :, N_LAYERS, :],
                    axis=AX.X,
                    op=OP.add,
                )
                for k in range(KC):
                    nc.vector.tensor_scalar(
                        out=hcol[k][:, s : s + 1],
                        in0=hred[:, k : k + 1],
                        scalar1=1.0 / T,
                        scalar2=None,
                        op0=OP.mult,
                    )
                nc.vector.tensor_reduce(
                    out=habs2[:, s : s + 1],
                    in_=hred,
                    axis=AX.X,
                    op=OP.add,
                    apply_absolute_value=True,
                )
            ps_x1 = poolps2.tile([1, SPC], f32, name="ps_x1", tag="misc")
            nc.tensor.matmul(ps_x1, ones128, habs2, start=True, stop=True)
            xs1 = const.tile([1, SPC], f32, name="xs1")
            nc.vector.tensor_scalar(
                out=xs1, in0=ps_x1, scalar1=1.0 / (C * T), scalar2=None, op0=OP.mult
            )
            ps_xb = poolps2.tile([128, SPC], f32, name="ps_xb", tag="misc")
            nc.tensor.matmul(ps_xb, ones_r, xs1, start=True, stop=True)
            xs1b = const.tile([128, SPC], f32, name="xs1b")
            nc.vector.tensor_copy(xs1b, ps_xb)

            h2 = [const.tile([128, SPC], bf16, name=f"h2_{mt}") for mt in range(2)]
            habs_h = [const.tile([128, SPC], f32, name=f"habs_h_{mt}") for mt in range(2)]
            ps_x2 = poolps2.tile([1, SPC], f32, name="ps_x2", tag="misc")
            for mt in range(2):
                ps_h = poolps2.tile([128, SPC], f32, name=f"ps_h_{mt}", tag="misc")
                for kc in range(KC):
                    nc.tensor.matmul(
                        ps_h,
                        w1t_sb[:, kc, mt * 128 : (mt + 1) * 128],
                        hcol[kc],
                        start=(kc == 0),
                        stop=(kc == 3),
                    )
                bt = const.tile([128, SPC], f32, name=f"bt_{mt}")
                nc.vector.tensor_scalar(
                    out=bt, in0=xs1b, scalar1=b1c_sb[:, mt : mt + 1], scalar2=None, op0=OP.mult
                )
                nc.vector.scalar_tensor_tensor(
                    out=h2[mt], in0=ps_h, scalar=c1, in1=bt, op0=OP.mult, op1=OP.add
                )
                nc.scalar.activation(
                    out=h2[mt], in_=h2[mt], func=AF.Prelu, bias=0.0, scale=1.0,
                    alpha=alpha_sb[:, 25:26],
                )
                nc.scalar.activation(out=habs_h[mt], in_=h2[mt], func=AF.Abs)
                nc.tensor.matmul(ps_x2, ones128, habs_h[mt], start=(mt == 0), stop=(mt == 1))
            xs2 = const.tile([1, SPC], f32, name="xs2")
            nc.vector.tensor_scalar(out=xs2, in0=ps_x2, scalar1=1.0 / 256, scalar2=None, op0=OP.mult)
            ps_o = poolps2.tile([1, SPC], f32, name="ps_o", tag="misc")
            for mt in range(2):
                nc.tensor.matmul(
                    ps_o, w2t_sb[:, mt : mt + 1], h2[mt], start=(mt == 0), stop=(mt == 1)
                )
            pre = const.tile([1, SPC], f32, name="pre")
            nc.vector.tensor_scalar(out=pre, in0=xs2, scalar1=b2c, scalar2=None, op0=OP.mult)
            pre2 = const.tile([1, SPC], f32, name="pre2")
            nc.vector.scalar_tensor_tensor(
                out=pre2, in0=ps_o, scalar=c2, in1=pre, op0=OP.mult, op1=OP.add
            )
            score = const.tile([1, SPC], f32, name="score")
            nc.scalar.activation(out=score, in_=pre2, func=AF.Sigmoid)
            nc.sync.dma_start(out=out, in_=score)
            if K_DEBUG:
                nc.sync.dma_start(out=dbg_pre, in_=pre2)
                for k in range(KC):
                    nc.sync.dma_start(out=dbg_h[:, k, :], in_=hcol[k])

    nc.compile()
    return nc


def _get_nc(baked):
    key = hashlib.sha256(
        (repr(baked) + f"|{N_LAYERS}|{SKIP_ENC}|{K_DEBUG}|{K_SUMSQ}|{K_RESID}|{K_PW}").encode()
    ).hexdigest()
    if key not in _CACHE:
        _CACHE[key] = _build(baked)
    return _CACHE[key]


def _in_maps(x, arrays):
    maps = []
    for c in range(NCORES):
        m = dict(arrays)
        m["x2"] = _f32(x[c * SPC : (c + 1) * SPC, 0, :])
        maps.append(m)
    return maps


def _gather(res):
    scores = np.empty((B, 1), dtype=np.float32)
    for c in range(NCORES):
        o = res.results[c]["out"]  # (1, SPC)
        for s in range(SPC):
            scores[c * SPC + s, 0] = o[0, s]
    return scores


def kernel(**inputs) -> np.ndarray:
    from concourse.bass_utils import run_bass_kernel_spmd

    x, arrays, baked = _prep(inputs)
    nc = _get_nc(baked)
    res = run_bass_kernel_spmd(nc, _in_maps(x, arrays), core_ids=list(range(NCORES)))
    return _gather(res)


def run_profiled(inputs, tmpdir=None):
    import ntff_shim

    ntff_shim.install()
    from concourse.bass_utils import run_bass_kernel_spmd

    x, arrays, baked = _prep(inputs)
    nc = _get_nc(baked)
    res = run_bass_kernel_spmd(
        nc, _in_maps(x, arrays), core_ids=list(range(NCORES)), trace=True, tmpdir=tmpdir
    )
    return _gather(res), res


# revision 19
# speedup vs baseline: 1.7052x; 1.0037x over previous
"""Trainium2 Bass kernel for nn_APDIntelligibilityEstimator.

Model: audio encoder conv(k=40,s=20) -> GroupNorm(1)+PReLU -> two 1x1
BitConv (fused into one 512x512 int matmul on host) -> 24 depthwise-separable
TCN blocks (dconv k=3 dil 2^(i%8) -> GN+PReLU -> 1x1 BitConv -> residual)
-> global avg pool -> BitLinear -> PReLU -> BitLinear -> sigmoid.

Sharding: data-parallel over batch, 2 samples per core on 8 NeuronCores.

Host-side weight preprocessing exploits exact algebraic identities of the
reference (x_scale cancellation in bit ops; composition of the two 1x1 sign
convs into one integer matrix; folding of w_scale*scale into the GN
affine+PReLU pass).

Device mapping per TCN layer:
  - depthwise side taps (t-d, t+d) -> TensorE diagonal matmuls into PSUM
  - center tap + combine           -> DVE scalar_tensor_tensor from PSUM
  - sum(g) for GN mean             -> edge-corrected from residual accum_out
                                      (no full pass)
  - sum(g^2) for GN var            -> ScalarE Square pass with accum_out
  - GN affine + PReLU + w_scale    -> one ScalarE Prelu pass (per-channel
                                      scale/bias APs)
  - 1x1 conv (+-1 weights, bf16)   -> TensorE matmuls
  - residual add + pooled sums     -> DVE scalar_tensor_tensor with accum_out
"""

import hashlib
import os
import sys

import numpy as np

sys.path.insert(0, os.path.dirname(os.path.abspath(__file__)))

N_TCN = 24
DILATIONS = [2 ** (i % 8) for i in range(N_TCN)]
B, T_IN = 16, 64000
T = 3201           # conv output length
TE = 3202          # allocated elementwise width (col T stays zero)
PAD = 128          # max dilation; zero padding on both sides of f
FW = PAD + TE + PAD
C = 512
KC = 4             # channel chunks of 128
CT = C * T
EPS = 1e-5
NCORES = 8
SPC = 2            # samples per core
PW = int(os.environ.get("K_PWW", "1536"))   # psum macro-tile width
_spans = []
_o = 0
while _o < T:
    _spans.append((_o, min(PW, T - _o)))
    _o += PW
NT_SPANS = _spans
NPSB = int(os.environ.get("K_NPSB", "2"))    # psum mm-pool bufs

N_LAYERS = int(os.environ.get("K_NLAYERS", str(N_TCN)))  # debug knob
SKIP_ENC = os.environ.get("K_SKIPENC", "0") == "1"        # debug knob
K_DEBUG = os.environ.get("K_DEBUG", "0") == "1"           # debug knob
K_SUMSQ = os.environ.get("K_SUMSQ", "act")                # act | dve
K_RESID = os.environ.get("K_RESID", "dve")                # dve | act
K_PW = os.environ.get("K_PW", "bf16")                     # bf16 | fp8
ATE = 3216         # fp8 act inner width (16-byte aligned)


def _f32(a):
    return np.ascontiguousarray(a, dtype=np.float32)


def _bf16(a):
    import ml_dtypes

    return np.ascontiguousarray(np.asarray(a, dtype=np.float32).astype(ml_dtypes.bfloat16))


def _prep(inp):
    """Host-side weight preprocessing. Returns (x, arrays, baked_scalars)."""
    x = np.asarray(inp["x"], dtype=np.float32)          # (16,1,64000)
    enc_w = np.asarray(inp["enc_w"], dtype=np.float32)  # (512,1,40)
    ew = enc_w[:, 0, :]
    encw = np.stack([ew[:, :20].T, ew[:, 20:].T])        # (2,20,512)

    sign = np.sign
    bn_w = np.asarray(inp["bn_w"], dtype=np.float32)[:, :, 0]    # (256,512)
    ti_w = np.asarray(inp["ti_w"], dtype=np.float32)[:, :, 0]    # (512,256)
    wbt = (sign(ti_w) @ sign(bn_w)).T                            # (512,512) ints
    cbt = float(
        np.abs(bn_w).mean() * inp["bn_scale"][0] * np.abs(ti_w).mean() * inp["ti_scale"][0]
    )

    pw_w = np.asarray(inp["pw_w"], dtype=np.float32)[:, :, :, 0]  # (24,512,512)
    c_pw = np.abs(pw_w).mean(axis=(1, 2)) * np.asarray(inp["pw_scale"], dtype=np.float32)[:, 0]
    assert np.all(c_pw > 0)
    if K_PW == "fp8":
        # exact +-1 weights in fp8; c_pw applied at the residual op
        pwt = sign(pw_w).transpose(0, 2, 1)                       # (24,512,512) [in,out]
    else:
        # fold w_scale*scale into the +-1 weights (+-c, exact-ish in bf16)
        pwt = (sign(pw_w) * c_pw[:, None, None]).transpose(0, 2, 1)

    dw_w = np.asarray(inp["dw_w"], dtype=np.float32)[:, :, 0, :]  # (24,512,3)
    dwc = dw_w.reshape(24, KC, 128, 3).transpose(2, 0, 1, 3)      # (128,24,4,3)
    # diagonal weight matrices for side taps on TensorE: (24, 2, KC, 128, 128)
    dwdiag = np.zeros((N_TCN, 2, KC, 128, 128), dtype=np.float32)
    idx = np.arange(128)
    for i in range(N_TCN):
        for jt, tap in enumerate((0, 2)):
            for k in range(KC):
                dwdiag[i, jt, k, idx, idx] = dw_w[i, k * 128 : (k + 1) * 128, tap]

    tg = np.asarray(inp["tcn_gamma"], dtype=np.float32)  # (24,512)
    tb = np.asarray(inp["tcn_beta"], dtype=np.float32)
    gbf = np.stack(
        [
            tg.reshape(24, KC, 128).transpose(2, 0, 1),
            tb.reshape(24, KC, 128).transpose(2, 0, 1),
        ],
        axis=-1,
    )  # (128,24,4,2)

    encgb = np.stack(
        [
            np.asarray(inp["enc_gamma"], dtype=np.float32).reshape(KC, 128).T,
            np.asarray(inp["enc_beta"], dtype=np.float32).reshape(KC, 128).T,
        ],
        axis=-1,
    )  # (128,4,2)

    alph = np.concatenate(
        [
            np.atleast_1d(np.float32(inp["enc_prelu"])),
            np.asarray(inp["tcn_prelu"], dtype=np.float32),
            np.atleast_1d(np.float32(inp["head_prelu"])),
        ]
    )[None, :]  # (1,26)

    fc1_w = np.asarray(inp["fc1_w"], dtype=np.float32)   # (256,512)
    c1 = float(np.abs(fc1_w).mean() * inp["fc1_scale"][0])
    w1t = sign(fc1_w).T                                   # (512,256)
    b1c = (np.asarray(inp["fc1_b"], dtype=np.float32) * c1).reshape(2, 128).T  # (128,2)

    fco_w = np.asarray(inp["fco_w"], dtype=np.float32)   # (1,256)
    c2 = float(np.abs(fco_w).mean() * inp["fco_scale"][0])
    w2t = sign(fco_w).T                                   # (256,1)
    b2c = float(np.asarray(inp["fco_b"], dtype=np.float32)[0] * c2)

    def _fp8(a):
        import ml_dtypes
        return np.ascontiguousarray(np.asarray(a, dtype=np.float32).astype(ml_dtypes.float8_e4m3))

    arrays = dict(
        encw=_bf16(encw),
        wbt=_bf16(wbt),
        pwt=_fp8(pwt) if K_PW == "fp8" else _bf16(pwt),
        dwdiag=_bf16(dwdiag),
        dwc=_f32(dwc),
        gbf=_f32(gbf),
        encgb=_f32(encgb),
        alph=_f32(alph),
        w1t=_bf16(w1t),
        w2t=_bf16(w2t),
        b1c=_f32(b1c),
    )
    baked = (cbt, c1, b2c, float(c2), tuple(float(v) for v in c_pw))
    return x, arrays, baked


# ---------------------------------------------------------------------------

_CACHE = {}


def _build(baked):
    assert not (K_PW == "fp8" and K_RESID == "act")
    import concourse.bacc as bacc
    import concourse.bass as bass
    import concourse.mybir as mybir
    import concourse.tile as tile

    cbt, c1, b2c, c2, c_pw = baked
    f32 = mybir.dt.float32
    bf16 = mybir.dt.bfloat16
    AF = mybir.ActivationFunctionType
    OP = mybir.AluOpType
    AX = mybir.AxisListType

    nc = bacc.Bacc("TRN2", target_bir_lowering=False, debug=False)

    x2 = nc.dram_tensor("x2", [SPC, T_IN], f32, kind="ExternalInput").ap()
    encw = nc.dram_tensor("encw", [2, 20, C], bf16, kind="ExternalInput").ap()
    wbt = nc.dram_tensor("wbt", [C, C], bf16, kind="ExternalInput").ap()
    fp8 = mybir.dt.float8e4
    pw_dt = fp8 if K_PW == "fp8" else bf16
    pwt = nc.dram_tensor("pwt", [N_TCN, C, C], pw_dt, kind="ExternalInput").ap()
    dwdiag = nc.dram_tensor(
        "dwdiag", [N_TCN, 2, KC, 128, 128], bf16, kind="ExternalInput"
    ).ap()
    dwc = nc.dram_tensor("dwc", [128, N_TCN, KC, 3], f32, kind="ExternalInput").ap()
    gbf = nc.dram_tensor("gbf", [128, N_TCN, KC, 2], f32, kind="ExternalInput").ap()
    encgb = nc.dram_tensor("encgb", [128, KC, 2], f32, kind="ExternalInput").ap()
    alph = nc.dram_tensor("alph", [1, 26], f32, kind="ExternalInput").ap()
    w1t = nc.dram_tensor("w1t", [C, 256], bf16, kind="ExternalInput").ap()
    w2t = nc.dram_tensor("w2t", [256, 1], bf16, kind="ExternalInput").ap()
    b1c = nc.dram_tensor("b1c", [128, 2], f32, kind="ExternalInput").ap()
    out = nc.dram_tensor("out", [1, SPC], f32, kind="ExternalOutput").ap()
    if K_DEBUG:
        dbg_f = nc.dram_tensor("dbg_f", [KC, 128, FW], bf16, kind="ExternalOutput").ap()
        dbg_g = nc.dram_tensor("dbg_g", [KC, 128, TE], bf16, kind="ExternalOutput").ap()
        dbg_h = nc.dram_tensor("dbg_h", [128, KC, SPC], bf16, kind="ExternalOutput").ap()
        dbg_pre = nc.dram_tensor("dbg_pre", [1, SPC], f32, kind="ExternalOutput").ap()

    ident_d = nc.inline_tensor(np.eye(128, dtype=np.float32), name="ident").ap()

    with tile.TileContext(nc) as tc:
        import contextlib

        ctx = contextlib.ExitStack()
        with ctx:
            const = ctx.enter_context(tc.tile_pool(name="const", bufs=1))
            poolf = ctx.enter_context(tc.tile_pool(name="poolf", bufs=1))
            poolg = ctx.enter_context(tc.tile_pool(name="poolg", bufs=1))
            poolw = ctx.enter_context(tc.tile_pool(name="poolw", bufs=3))
            poolscr = ctx.enter_context(tc.tile_pool(name="poolscr", bufs=3))
            poolst = ctx.enter_context(tc.tile_pool(name="poolst", bufs=3))
            poolps = ctx.enter_context(tc.tile_pool(name="poolps", bufs=NPSB, space="PSUM"))
            poolps2 = ctx.enter_context(tc.tile_pool(name="poolps2", bufs=2, space="PSUM"))

            # ---- constants / weights resident in SBUF ----
            alpha_sb = const.tile([128, 26], f32, name="alpha_sb")
            nc.sync.dma_start(
                out=alpha_sb,
                in_=bass.AP(tensor=alph.tensor, offset=alph.offset, ap=[[0, 128], [1, 26]]),
            )
            dwc_sb = const.tile([128, N_TCN, KC, 3], f32, name="dwc_sb")
            nc.sync.dma_start(out=dwc_sb, in_=dwc)
            gbf_sb = const.tile([128, N_TCN, KC, 2], f32, name="gbf_sb")
            nc.sync.dma_start(out=gbf_sb, in_=gbf)
            encgb_sb = const.tile([128, KC, 2], f32, name="encgb_sb")
            nc.sync.dma_start(out=encgb_sb, in_=encgb)
            enca = const.tile([20, C], bf16, name="enca")
            nc.sync.dma_start(out=enca, in_=encw[0])
            encb = const.tile([20, C], bf16, name="encb")
            nc.sync.dma_start(out=encb, in_=encw[1])
            wbt_sb = const.tile([128, KC, C], bf16, name="wbt_sb")
            nc.sync.dma_start(out=wbt_sb, in_=wbt.rearrange("(kc kp) m -> kp kc m", kp=128))
            w1t_sb = const.tile([128, KC, 256], bf16, name="w1t_sb")
            nc.sync.dma_start(out=w1t_sb, in_=w1t.rearrange("(kc kp) m -> kp kc m", kp=128))
            w2t_sb = const.tile([128, 2], bf16, name="w2t_sb")
            nc.sync.dma_start(out=w2t_sb, in_=w2t.rearrange("(mt kp) o -> kp (mt o)", kp=128))
            b1c_sb = const.tile([128, 2], f32, name="b1c_sb")
            nc.sync.dma_start(out=b1c_sb, in_=b1c)
            ident = const.tile([128, 128], f32, name="ident")
            nc.sync.dma_start(out=ident, in_=ident_d)
            identb = const.tile([128, 128], bf16, name="identb")
            nc.vector.tensor_copy(identb, ident)
            ones128 = const.tile([128, 1], f32, name="ones128")
            nc.vector.memset(ones128, 1.0)
            ones_r = const.tile([1, 128], f32, name="ones_r")
            nc.vector.memset(ones_r, 1.0)
            eps_sb = const.tile([128, 1], f32, name="eps_sb")
            nc.vector.memset(eps_sb, EPS)

            # ---- persistent activation state ----
            f = [
                [poolf.tile([128, FW], bf16, name=f"f_{s}_{k}", tag=f"f_{s}_{k}") for k in range(KC)]
                for s in range(SPC)
            ]
            g = [
                poolg.tile([128, KC, TE], bf16, name=f"g_{s}", tag=f"g_{s}")
                for s in range(SPC)
            ]
            act8 = None
            if K_PW == "fp8":
                act8 = [
                    poolg.tile([128, KC, ATE], fp8, name=f"act8_{s}", tag=f"act8_{s}")
                    for s in range(SPC)
                ]
            for s in range(SPC):
                for k in range(KC):
                    nc.gpsimd.memset(f[s][k], 0.0)
                nc.gpsimd.memset(g[s], 0.0)

            # pooled-sum accumulator slots: [bnti, layer0..23] x 3 spans
            hacc = [
                poolst.tile(
                    [128, KC, N_TCN + 1, len(NT_SPANS)], f32, name=f"hacc_{s}", tag=f"hacc{s}", bufs=1
                )
                for s in range(SPC)
            ]

            if SKIP_ENC:
                for s in range(SPC):
                    nc.vector.memset(hacc[s], 0.0)

            # ---------------- helpers ----------------
            def stat_chain(s, S_ap, Q_ap, gb0, gb1, alpha_idx):
                """Emit scalar chain: rs = 1/sqrt(var+eps) (+1 Newton step) and
                the (128,KC) scale/bias tiles for the fused GN+PReLU pass."""
                mt_ = poolst.tile([1, 1], f32, name=f"mt_{s}", tag=f"mt{s}")
                nc.vector.tensor_scalar(out=mt_, in0=S_ap, scalar1=1.0 / CT, scalar2=None, op0=OP.mult)
                negmsq = poolst.tile([1, 1], f32, name=f"negmsq_{s}", tag=f"negmsq{s}")
                nc.vector.tensor_scalar(
                    out=negmsq, in0=mt_, scalar1=mt_, scalar2=-1.0, op0=OP.mult, op1=OP.mult
                )
                vt = poolst.tile([1, 1], f32, name=f"vt_{s}", tag=f"vt{s}")
                nc.vector.scalar_tensor_tensor(
                    out=vt, in0=Q_ap, scalar=1.0 / CT, in1=negmsq, op0=OP.mult, op1=OP.add
                )
                std = poolst.tile([1, 1], f32, name=f"std_{s}", tag=f"std{s}")
                nc.scalar.activation(out=std, in_=vt, func=AF.Sqrt, bias=eps_sb[:1], scale=1.0)
                r0 = poolst.tile([1, 1], f32, name=f"r0_{s}", tag=f"r0{s}")
                nc.vector.reciprocal(out=r0, in_=std)
                tn = poolst.tile([1, 1], f32, name=f"tn_{s}", tag=f"tn{s}")
                nc.vector.tensor_mul(tn, std, r0)
                nc.vector.tensor_scalar(out=tn, in0=tn, scalar1=-1.0, scalar2=2.0, op0=OP.mult, op1=OP.add)
                rsn = poolst.tile([1, 2], f32, name=f"rsn_{s}", tag=f"rsn{s}")
                nc.vector.tensor_mul(rsn[:, 0:1], r0, tn)
                nc.vector.tensor_scalar(
                    out=rsn[:, 1:2], in0=mt_, scalar1=rsn[:, 0:1], scalar2=-1.0, op0=OP.mult, op1=OP.mult
                )
                ps_bc = poolps2.tile([128, 2], f32, name=f"psbc_{s}", tag="misc")
                nc.tensor.matmul(ps_bc, ones_r, rsn, start=True, stop=True)
                bc = poolst.tile([128, 2], f32, name=f"bc_{s}", tag=f"bc{s}")
                nc.vector.tensor_copy(bc, ps_bc)
                sc_ = poolst.tile([128, KC], f32, name=f"sc_{s}", tag=f"sc{s}")
                nc.vector.tensor_scalar(out=sc_, in0=gb0, scalar1=bc[:, 0:1], scalar2=None, op0=OP.mult)
                sb_ = poolst.tile([128, KC], f32, name=f"sb_{s}", tag=f"sb{s}")
                nc.vector.scalar_tensor_tensor(
                    out=sb_, in0=gb0, scalar=bc[:, 1:2], in1=gb1, op0=OP.mult, op1=OP.add
                )
                return sc_, sb_, alpha_sb[:, alpha_idx : alpha_idx + 1]

            def prelu_pass(s, sc_, sb_, alpha_ap, to_act8=False):
                for k in range(KC):
                    dst = act8[s][:, k, :T] if to_act8 else g[s][:, k, :T]
                    nc.scalar.activation(
                        out=dst,
                        in_=g[s][:, k, :T],
                        func=AF.Prelu,
                        bias=sb_[:, k : k + 1],
                        scale=sc_[:, k : k + 1],
                        alpha=alpha_ap,
                    )

            # ================= encoder =================
            rhs20 = []
            if not SKIP_ENC:
              with tc.tile_pool(name="poolenc", bufs=1) as poolenc, tc.tile_pool(
                name="poolxc", bufs=3
              ) as poolxc:
                for s in range(SPC):
                    r = poolenc.tile([20, TE], bf16, name=f"rhs20_{s}", tag=f"rhs20_{s}")
                    rhs20.append(r)
                    nc.gpsimd.memset(r, 0.0)
                    for jg in range(7):  # groups of 4 chunks of 128 windows
                        ps_t = poolps2.tile([20, 512], f32, name=f"ps_t_{s}_{jg}", tag="misc")
                        width = 0
                        for jj in range(4):
                            j = jg * 4 + jj
                            if j >= 25:
                                break
                            nwin = min(128, 3200 - j * 128)
                            xc = poolxc.tile([128, 20], f32, name=f"xc_{s}_{j}", tag="xc")
                            nc.sync.dma_start(
                                out=xc[:nwin],
                                in_=x2[s, j * 2560 : j * 2560 + nwin * 20].rearrange(
                                    "(r k) -> r k", k=20
                                ),
                            )
                            nc.tensor.transpose(ps_t[:, jj * 128 : jj * 128 + nwin], xc[:nwin], ident)
                            width += nwin
                        nc.scalar.copy(
                            out=r[:, 1 + jg * 512 : 1 + jg * 512 + width], in_=ps_t[:, :width]
                        )

                eacc = [
                    poolst.tile([128, 32], f32, name=f"eacc_{s}", tag=f"eacc{s}", bufs=1)
                    for s in range(SPC)
                ]
                for s in range(SPC):
                    for m in range(KC):
                        for nt, (n0, w) in enumerate(NT_SPANS):
                            ps = poolps.tile([128, PW], f32, name=f"ps_e_{s}_{m}_{nt}", tag="mm")
                            for sub in range(0, w, 512):
                                nn = min(512, w - sub)
                                o = n0 + sub
                                nc.tensor.matmul(
                                    ps[:, sub : sub + nn],
                                    enca[:, m * 128 : (m + 1) * 128],
                                    rhs20[s][:, o : o + nn],
                                    start=True,
                                    stop=False,
                                )
                                nc.tensor.matmul(
                                    ps[:, sub : sub + nn],
                                    encb[:, m * 128 : (m + 1) * 128],
                                    rhs20[s][:, o + 1 : o + 1 + nn],
                                    start=False,
                                    stop=True,
                                )
                                nc.vector.tensor_scalar(
                                    out=g[s][:, m, o : o + nn],
                                    in0=ps[:, sub : sub + nn],
                                    scalar1=1.0,
                                    scalar2=None,
                                    op0=OP.mult,
                                    op1=OP.add,
                                    accum_out=eacc[s][:, m * 7 + (n0 + sub) // 512 : m * 7 + (n0 + sub) // 512 + 1],
                                )
                    for m in range(KC):
                        scr = poolscr.tile([128, TE], bf16, name=f"scr_e_{s}_{m}", tag="gsq")
                        nc.vector.scalar_tensor_tensor(
                            out=scr[:, :T],
                            in0=g[s][:, m, :T],
                            scalar=1.0,
                            in1=g[s][:, m, :T],
                            op0=OP.mult,
                            op1=OP.mult,
                            accum_out=eacc[s][:, 28 + m : 29 + m],
                        )

                for s in range(SPC):
                    ps_st = poolps2.tile([1, 32], f32, name=f"ps_ste_{s}", tag="misc")
                    nc.tensor.matmul(ps_st, ones128, eacc[s], start=True, stop=True)
                    SQ = poolst.tile([1, 2], f32, name=f"SQe_{s}", tag=f"SQ{s}")
                    nc.vector.tensor_reduce(
                        out=SQ[:, 0:1], in_=ps_st[:, 0:28], axis=AX.X, op=OP.add
                    )
                    nc.vector.tensor_reduce(
                        out=SQ[:, 1:2], in_=ps_st[:, 28:32], axis=AX.X, op=OP.add
                    )
                    sc_, sb_, a_ = stat_chain(
                        s, SQ[:, 0:1], SQ[:, 1:2], encgb_sb[:, :, 0], encgb_sb[:, :, 1], 0
                    )
                    prelu_pass(s, sc_, sb_, a_)

                # fused bottleneck+tcn_input 1x1 (integer matrix) -> f0
                for s in range(SPC):
                    for m in range(KC):
                        for nt, (n0, w) in enumerate(NT_SPANS):
                            ps = poolps.tile([128, PW], f32, name=f"ps_bt_{s}_{m}_{nt}", tag="mm")
                            for sub in range(0, w, 512):
                                nn = min(512, w - sub)
                                o = n0 + sub
                                for kc in range(KC):
                                    nc.tensor.matmul(
                                        ps[:, sub : sub + nn],
                                        wbt_sb[:, kc, m * 128 : (m + 1) * 128],
                                        g[s][:, kc, o : o + nn],
                                        start=(kc == 0),
                                        stop=(kc == 3),
                                    )
                            nc.vector.tensor_scalar(
                                out=f[s][m][:, PAD + n0 : PAD + n0 + w],
                                in0=ps[:, :w],
                                scalar1=cbt,
                                scalar2=None,
                                op0=OP.mult,
                                op1=OP.add,
                                accum_out=hacc[s][:, m, 0, nt : nt + 1],
                            )

            # ================= TCN layers =================
            for i in range(N_LAYERS):
                d = DILATIONS[i]
                wtile = poolw.tile([128, KC, C], pw_dt, name=f"wt_{i}", tag="pw")
                nc.sync.dma_start(out=wtile, in_=pwt[i].rearrange("(kc kp) m -> kp kc m", kp=128))
                dtile = poolw.tile([128, 2, KC, 128], bf16, name=f"dt_{i}", tag="dw")
                nc.sync.dma_start(
                    out=dtile, in_=dwdiag[i].rearrange("j k p m -> p j k m")
                )
                w0c = dwc_sb[:, i, :, 0]
                w1c = dwc_sb[:, i, :, 1]
                w2c = dwc_sb[:, i, :, 2]
                accs = []
                # ---- sum(g) via edge-corrected residual sums (no pass) ----
                for s in range(SPC):
                    acc = poolst.tile([128, 8], f32, name=f"acc_{i}_{s}", tag=f"acc{s}")
                    accs.append(acc)
                    S_t = poolst.tile([128, KC], f32, name=f"S_{i}_{s}", tag=f"S{s}")
                    nc.vector.tensor_reduce(
                        out=S_t, in_=hacc[s][:, :, i, :], axis=AX.X, op=OP.add
                    )
                    ee = poolst.tile([128, 2, KC], f32, name=f"ee_{i}_{s}", tag=f"ee{s}")
                    for k in range(KC):
                        nc.vector.tensor_reduce(
                            out=ee[:, 0, k : k + 1], in_=f[s][k][:, PAD : PAD + d],
                            axis=AX.X, op=OP.add,
                        )
                        nc.vector.tensor_reduce(
                            out=ee[:, 1, k : k + 1], in_=f[s][k][:, PAD + T - d : PAD + T],
                            axis=AX.X, op=OP.add,
                        )
                    t1 = poolst.tile([128, KC], f32, name=f"t1_{i}_{s}", tag=f"t1{s}")
                    nc.vector.tensor_sub(t1, S_t, ee[:, 1, :])   # S - Er
                    nc.vector.tensor_mul(t1, t1, w0c)
                    t2 = poolst.tile([128, KC], f32, name=f"t2_{i}_{s}", tag=f"t2{s}")
                    nc.vector.tensor_sub(t2, S_t, ee[:, 0, :])   # S - El
                    nc.vector.tensor_mul(t2, t2, w2c)
                    nc.vector.tensor_add(t1, t1, t2)
                    t3 = poolst.tile([128, KC], f32, name=f"t3_{i}_{s}", tag=f"t3{s}")
                    nc.vector.tensor_mul(t3, S_t, w1c)
                    nc.vector.tensor_add(acc[:, 0:4], t1, t3)

                # ---- depthwise conv: side taps on PE, combine on DVE ----
                tps = {}
                for s in range(SPC):
                    for k in range(KC):
                        for nt, (n0, w) in enumerate(NT_SPANS):
                            tp = poolps.tile([128, PW], f32, name=f"tp_{i}_{s}_{k}_{nt}", tag="mm")
                            tps[(s, k, nt)] = tp
                            for sub in range(0, w, 512):
                                nn = min(512, w - sub)
                                o = PAD + n0 + sub
                                nc.tensor.matmul(
                                    tp[:, sub : sub + nn],
                                    dtile[:, 0, k, :],
                                    f[s][k][:, o - d : o - d + nn],
                                    start=True,
                                    stop=False,
                                )
                                nc.tensor.matmul(
                                    tp[:, sub : sub + nn],
                                    dtile[:, 1, k, :],
                                    f[s][k][:, o + d : o + d + nn],
                                    start=False,
                                    stop=True,
                                )
                            nc.vector.scalar_tensor_tensor(
                                out=g[s][:, k, n0 : n0 + w],
                                in0=f[s][k][:, PAD + n0 : PAD + n0 + w],
                                scalar=w1c[:, k : k + 1],
                                in1=tp[:, :w],
                                op0=OP.mult,
                                op1=OP.add,
                            )
                # ---- sum(g^2) ----
                for s in range(SPC):
                    for k in range(KC):
                        scr = poolscr.tile([128, TE], bf16, name=f"scr_{i}_{s}_{k}", tag="gsq")
                        if K_SUMSQ == "act":
                            nc.scalar.activation(
                                out=scr[:, :T],
                                in_=g[s][:, k, :T],
                                func=AF.Square,
                                accum_out=accs[s][:, 4 + k : 5 + k],
                            )
                        else:
                            nc.vector.scalar_tensor_tensor(
                                out=scr[:, :T],
                                in0=g[s][:, k, :T],
                                scalar=1.0,
                                in1=g[s][:, k, :T],
                                op0=OP.mult,
                                op1=OP.mult,
                                accum_out=accs[s][:, 4 + k : 5 + k],
                            )
                # ---- GN stats + PReLU ----
                for s in range(SPC):
                    ps_st = poolps2.tile([1, 8], f32, name=f"ps_st_{i}_{s}", tag="misc")
                    nc.tensor.matmul(ps_st, ones128, accs[s], start=True, stop=True)
                    SQ = poolst.tile([1, 2], f32, name=f"SQ_{i}_{s}", tag=f"SQ{s}")
                    nc.vector.tensor_reduce(
                        out=SQ,
                        in_=ps_st.rearrange("p (a b) -> p a b", a=2),
                        axis=AX.X,
                        op=OP.add,
                    )
                    sc_, sb_, a_ = stat_chain(
                        s, SQ[:, 0:1], SQ[:, 1:2], gbf_sb[:, i, :, 0], gbf_sb[:, i, :, 1], 1 + i
                    )
                    prelu_pass(s, sc_, sb_, a_, to_act8=(K_PW == "fp8"))
                # ---- pointwise 1x1 + residual (+ pooled-sum accum) ----
                for s in range(SPC):
                    for m in range(KC):
                        for nt, (n0, w) in enumerate(NT_SPANS):
                            ps = poolps.tile([128, PW], f32, name=f"ps_{i}_{s}_{m}_{nt}", tag="mm")
                            for sub in range(0, w, 512):
                                nn = min(512, w - sub)
                                o = n0 + sub
                                if K_PW == "fp8":
                                    for kc in (0, 2):
                                        nc.tensor.matmul(
                                            ps[:, sub : sub + nn],
                                            wtile[:, kc : kc + 2, m * 128 : (m + 1) * 128],
                                            act8[s][:, kc : kc + 2, o : o + nn],
                                            start=(kc == 0),
                                            stop=(kc == 2 and K_RESID != "act"),
                                            perf_mode=mybir.MatmulPerfMode.DoubleRow,
                                        )
                                else:
                                    for kc in range(KC):
                                        nc.tensor.matmul(
                                            ps[:, sub : sub + nn],
                                            wtile[:, kc, m * 128 : (m + 1) * 128],
                                            g[s][:, kc, o : o + nn],
                                            start=(kc == 0),
                                            stop=(kc == 3 and K_RESID != "act"),
                                        )
                                if K_RESID == "act":
                                    nc.tensor.matmul(
                                        ps[:, sub : sub + nn],
                                        identb,
                                        f[s][m][:, PAD + o : PAD + o + nn],
                                        start=False,
                                        stop=True,
                                    )
                            fslice = f[s][m][:, PAD + n0 : PAD + n0 + w]
                            rscale = c_pw[i] if K_PW == "fp8" else 1.0
                            if K_RESID == "act":
                                nc.scalar.activation(
                                    out=fslice,
                                    in_=ps[:, :w],
                                    func=AF.Copy,
                                    accum_out=hacc[s][:, m, i + 1, nt : nt + 1],
                                )
                            else:
                                nc.vector.scalar_tensor_tensor(
                                    out=fslice,
                                    in0=ps[:, :w],
                                    scalar=rscale,
                                    in1=fslice,
                                    op0=OP.mult,
                                    op1=OP.add,
                                    accum_out=hacc[s][:, m, i + 1, nt : nt + 1],
                                )

            # ================= debug dumps =================
            if K_DEBUG:
                for k in range(KC):
                    nc.sync.dma_start(out=dbg_f[k], in_=f[0][k])
                    nc.sync.dma_start(out=dbg_g[k], in_=g[0][:, k, :])

            # ================= head =================
            hcol = [const.tile([128, SPC], bf16, name=f"hcol_{k}") for k in range(KC)]
            habs2 = const.tile([128, SPC], f32, name="habs2")
            for s in range(SPC):
                hred = poolst.tile([128, KC], f32, name=f"hred_{s}", tag=f"hred{s}")
                nc.vector.tensor_reduce(
                    out=hred,
                    in_=hacc[s][:, :, N_LAYERS, :],
                    axis=AX.X,
                    op=OP.add,
                )
                for k in range(KC):
                    nc.vector.tensor_scalar(
                        out=hcol[k][:, s : s + 1],
                        in0=hred[:, k : k + 1],
                        scalar1=1.0 / T,
                        scalar2=None,
                        op0=OP.mult,
                    )
                nc.vector.tensor_reduce(
                    out=habs2[:, s : s + 1],
                    in_=hred,
                    axis=AX.X,
                    op=OP.add,
                    apply_absolute_value=True,
                )
            ps_x1 = poolps2.tile([1, SPC], f32, name="ps_x1", tag="misc")
            nc.tensor.matmul(ps_x1, ones128, habs2, start=True, stop=True)
            xs1 = const.tile([1, SPC], f32, name="xs1")
            nc.vector.tensor_scalar(
                out=xs1, in0=ps_x1, scalar1=1.0 / (C * T), scalar2=None, op0=OP.mult
            )
            ps_xb = poolps2.tile([128, SPC], f32, name="ps_xb", tag="misc")
            nc.tensor.matmul(ps_xb, ones_r, xs1, start=True, stop=True)
            xs1b = const.tile([128, SPC], f32, name="xs1b")
            nc.vector.tensor_copy(xs1b, ps_xb)

            h2 = [const.tile([128, SPC], bf16, name=f"h2_{mt}") for mt in range(2)]
            habs_h = [const.tile([128, SPC], f32, name=f"habs_h_{mt}") for mt in range(2)]
            ps_x2 = poolps2.tile([1, SPC], f32, name="ps_x2", tag="misc")
            for mt in range(2):
                ps_h = poolps2.tile([128, SPC], f32, name=f"ps_h_{mt}", tag="misc")
                for kc in range(KC):
                    nc.tensor.matmul(
                        ps_h,
                        w1t_sb[:, kc, mt * 128 : (mt + 1) * 128],
                        hcol[kc],
                        start=(kc == 0),
                        stop=(kc == 3),
                    )
                bt = const.tile([128, SPC], f32, name=f"bt_{mt}")
                nc.vector.tensor_scalar(
                    out=bt, in0=xs1b, scalar1=b1c_sb[:, mt : mt + 1], scalar2=None, op0=OP.mult
                )
                nc.vector.scalar_tensor_tensor(
                    out=h2[mt], in0=ps_h, scalar=c1, in1=bt, op0=OP.mult, op1=OP.add
                )
                nc.scalar.activation(
                    out=h2[mt], in_=h2[mt], func=AF.Prelu, bias=0.0, scale=1.0,
                    alpha=alpha_sb[:, 25:26],
                )
                nc.scalar.activation(out=habs_h[mt], in_=h2[mt], func=AF.Abs)
                nc.tensor.matmul(ps_x2, ones128, habs_h[mt], start=(mt == 0), stop=(mt == 1))
            xs2 = const.tile([1, SPC], f32, name="xs2")
            nc.vector.tensor_scalar(out=xs2, in0=ps_x2, scalar1=1.0 / 256, scalar2=None, op0=OP.mult)
            ps_o = poolps2.tile([1, SPC], f32, name="ps_o", tag="misc")
            for mt in range(2):
                nc.tensor.matmul(
                    ps_o, w2t_sb[:, mt : mt + 1], h2[mt], start=(mt == 0), stop=(mt == 1)
                )
            pre = const.tile([1, SPC], f32, name="pre")
            nc.vector.tensor_scalar(out=pre, in0=xs2, scalar1=b2c, scalar2=None, op0=OP.mult)
            pre2 = const.tile([1, SPC], f32, name="pre2")
            nc.vector.scalar_tensor_tensor(
                out=pre2, in0=ps_o, scalar=c2, in1=pre, op0=OP.mult, op1=OP.add
            )
            score = const.tile([1, SPC], f32, name="score")
            nc.scalar.activation(out=score, in_=pre2, func=AF.Sigmoid)
            nc.sync.dma_start(out=out, in_=score)
            if K_DEBUG:
                nc.sync.dma_start(out=dbg_pre, in_=pre2)
                for k in range(KC):
                    nc.sync.dma_start(out=dbg_h[:, k, :], in_=hcol[k])

    nc.compile()
    return nc


def _get_nc(baked):
    key = hashlib.sha256(
        (repr(baked) + f"|{N_LAYERS}|{SKIP_ENC}|{K_DEBUG}|{K_SUMSQ}|{K_RESID}|{K_PW}|{PW}|{NPSB}").encode()
    ).hexdigest()
    if key not in _CACHE:
        _CACHE[key] = _build(baked)
    return _CACHE[key]


def _in_maps(x, arrays):
    maps = []
    for c in range(NCORES):
        m = dict(arrays)
        m["x2"] = _f32(x[c * SPC : (c + 1) * SPC, 0, :])
        maps.append(m)
    return maps


def _gather(res):
    scores = np.empty((B, 1), dtype=np.float32)
    for c in range(NCORES):
        o = res.results[c]["out"]  # (1, SPC)
        for s in range(SPC):
            scores[c * SPC + s, 0] = o[0, s]
    return scores


def kernel(**inputs) -> np.ndarray:
    from concourse.bass_utils import run_bass_kernel_spmd

    x, arrays, baked = _prep(inputs)
    nc = _get_nc(baked)
    res = run_bass_kernel_spmd(nc, _in_maps(x, arrays), core_ids=list(range(NCORES)))
    return _gather(res)


def run_profiled(inputs, tmpdir=None):
    import ntff_shim

    ntff_shim.install()
    from concourse.bass_utils import run_bass_kernel_spmd

    x, arrays, baked = _prep(inputs)
    nc = _get_nc(baked)
    res = run_bass_kernel_spmd(
        nc, _in_maps(x, arrays), core_ids=list(range(NCORES)), trace=True, tmpdir=tmpdir
    )
    return _gather(res), res
